# revision 66
# baseline (speedup 1.0000x reference)
"""Trainium2 Bass kernel for decomposed-rel-pos attention (B=4, H=W=32, DIM=768, HEADS=12).

Sharding: 48 (batch, head) pairs -> 8 cores x 6 heads (core c: batch c//2,
heads (c%2)*6 .. +6). Each core computes qkv for its heads, attention with the
decomposed rel-pos bias folded into the S matmul as extra contraction rows
(0/1 expander matrices), softmax without max-subtraction, row-sums via a
ones-column appended to V, and a partial head-projection. Host sums the two
half-head partials per batch and adds an effective proj bias (which also
absorbs the v-bias exactly, since softmax rows sum to 1).

Numerics: all device tensors are fp16 (PE runs fp16 at 1 cycle/row with no
small-N penalty; PSUM accumulation is fp32). The softmax scale (1/8) is folded
into the exp activation's scale operand, with the rel-pos tables pre-scaled by
8 on host so the bias term comes out unscaled. End-to-end rel err vs the fp32
jax reference: ~1e-3 (tolerance 2e-2).

Layout/throughput notes (~102us/core cost-model estimate, vs 133us for the
previous f32r feature-major version):
- AV runs token-major (out [q,65] tiles, ap=65 per matmul; stationary = attnT
  slices): 4160 cycles/head + 1024 for the PE transpose back to feature-major,
  vs 8192 feature-major. Normalization becomes a per-partition scalar multiply
  (TensorScalarPtr) with one batched reciprocal per 4-chunk group.
- The pav accumulators are zeroed by a PE matmul against a zeros tile and all
  AV matmuls accumulate (start=False): interleaved start=True accumulation
  groups within one PSUM bank corrupt earlier regions on hardware.
- GPSIMD cannot access PSUM, so every PSUM-reading copy is on DVE (steady
  state) or Act (prologue/tail, where exp is not running).
- Inputs are host-packed into SBUF-layout [128, X] dram tensors so each loads
  with one large-descriptor DMA (10 input DMAs; HWDGE serialization is ~630ns
  per dma_start); xT is split in 3 so the qk projection starts early.
- Software pipeline: ladder(h) runs head h's S/exp stream (the pacer: 8 x
  ~1040ns exps on Act) and interleaves, as idle fillers: qk projection and
  rel tables for head h+2 (two ladders of slack on the single ps_a slot
  chain), the previous head's AV tail + normalize + transpose, and this
  head's AV units trailing the exps by 3.
- PSUM budget is exactly 8 banks: S pool 2x[128,1024], AV pool 2x[128,512]
  (pav accumulators then transpose targets), qk/rel-table pool 1x[128,1024].
"""
from contextlib import ExitStack

import numpy as np

import concourse.bass as bass
import concourse.bacc as bacc
import concourse.mybir as mybir
import concourse.tile as tile
from concourse.bass_utils import run_bass_kernel_spmd

B, H, W, DIM, HEADS = 4, 32, 32, 768, 12
HD = DIM // HEADS  # 64
N = H * W  # 1024
HPC = HEADS // 2  # heads per core = 6
NCORES = 8
F32 = mybir.dt.float32
F16 = mybir.dt.float16
EXPF = mybir.ActivationFunctionType.Exp
IDENTF = mybir.ActivationFunctionType.Identity

_cache = {}


def build_program(with_qk_bias=False):
    nc = bacc.Bacc("TRN2", target_bir_lowering=False, debug=False,
                   enable_asserts=False, num_devices=NCORES)
    xT = nc.dram_tensor("xT", [128, 6 * N], F16, kind="ExternalInput")
    wqk = nc.dram_tensor("wqk", [128, 6 * 768], F16, kind="ExternalInput")
    wv = nc.dram_tensor("wv", [128, 6 * 384], F16, kind="ExternalInput")
    wp = nc.dram_tensor("wp", [128, 3 * 768], F16, kind="ExternalInput")
    tbl = nc.dram_tensor("tbl", [128, 2304], F16, kind="ExternalInput")
    out_d = nc.dram_tensor("out_part", [N, DIM], F16, kind="ExternalOutput")

    with ExitStack() as ctx:
        tc = ctx.enter_context(tile.TileContext(nc))
        _body(nc, tc, ctx, xT, wqk, wv, wp, tbl, out_d, with_qk_bias)
    nc.compile()
    return nc


def _body(nc, tc, ctx, xT, wqk, wv, wp, tbl, out_d, with_qk_bias):
    persist = ctx.enter_context(tc.tile_pool(name="persist", bufs=1))
    attn_pool = ctx.enter_context(tc.tile_pool(name="attn", bufs=1))
    small = ctx.enter_context(tc.tile_pool(name="small", bufs=2))
    outp = ctx.enter_context(tc.tile_pool(name="outp", bufs=6))
    ps_s = ctx.enter_context(tc.tile_pool(name="ps_s", bufs=2, space="PSUM"))
    ps_av = ctx.enter_context(tc.tile_pool(name="ps_av", bufs=2, space="PSUM"))
    ps_a = ctx.enter_context(tc.tile_pool(name="ps_a", bufs=1, space="PSUM"))

    # ---- persistent SBUF tiles ----
    xT_sb = persist.tile([128, 6 * N], F16, tag="xt", name="xt")
    wqk_sb = persist.tile([128, 6 * 768], F16, tag="wqk", name="wqk")
    wv_sb = persist.tile([128, 6 * 384], F16, tag="wv", name="wv")
    wp_sb = persist.tile([128, 3 * 768], F16, tag="wp", name="wp")
    tbl_sb = persist.tile([128, 2304], F16, tag="tbl", name="tbl")
    rhT = tbl_sb[0:64, 0:1024]
    rwT = persist.tile([64, 1024], F16, tag="rwT", name="rwT")
    ecomb = tbl_sb[0:64, 1024:2048]
    ident = tbl_sb[:, 2048:2176]
    v_sb = [persist.tile([128, HPC * 65], F16, tag=f"v{m}", name=f"v{m}")
            for m in range(8)]
    proj_lhsT = [persist.tile([128, N], F16, tag=f"pl{t}", name=f"pl{t}")
                 for t in range(3)]
    lhsT_c = [persist.tile([128, N], F16, tag=f"lhs{i}", name=f"lhs{i}")
              for i in range(3)]
    rhs_c = [persist.tile([128, N], F16, tag=f"rhs{i}", name=f"rhs{i}")
             for i in range(3)]
    attnT = [attn_pool.tile([128, N], F16, tag=f"attnT{kb}", name=f"attnT{kb}")
             for kb in range(8)]
    zeros = persist.tile([128, 260], F16, tag="zeros", name="zeros")
    nc.vector.memset(zeros[:], 0.0)

    # ---- input DMAs: two queues, interleaved; each is one large transfer ----
    # wqk is packed by head-pair group g: col g*1536 + kc*256 + (h%2)*128 + c,
    # so group g arrives early enough to gate only heads 2g, 2g+1.
    nc.sync.dma_start(xT_sb[:, 0:N], xT[:, 0:N])
    nc.sync.dma_start(xT_sb[:, N:3 * N], xT[:, N:3 * N])
    nc.scalar.dma_start(wqk_sb[:, 0:1536], wqk[:, 0:1536])
    nc.scalar.dma_start(xT_sb[:, 3 * N:6 * N], xT[:, 3 * N:6 * N])
    nc.scalar.dma_start(tbl_sb[:, 0:2048], tbl[:, 0:2048])
    nc.scalar.dma_start(tbl_sb[:, 2048:2304], tbl[:, 2048:2304])
    nc.scalar.dma_start(wv_sb[:], wv[:])
    nc.scalar.dma_start(wqk_sb[:, 1536:3072], wqk[:, 1536:3072])
    nc.scalar.dma_start(wqk_sb[:, 3072:4608], wqk[:, 3072:4608])
    nc.scalar.dma_start(wp_sb[:], wp[:])

    # ecomb rows into both rhs buffers once (rows 64:128 never rewritten);
    # rwT to a partition-0 tile (PE matmul needs matching base partitions)
    nc.vector.tensor_copy(rwT[:], tbl_sb[64:128, 0:1024])
    nc.vector.tensor_copy(rhs_c[0][64:128, :], ecomb)
    nc.vector.tensor_copy(rhs_c[1][64:128, :], ecomb)
    nc.vector.tensor_copy(rhs_c[2][64:128, :], ecomb)

    def wqk_ap(kc, h):
        g, h2 = h // 2, h % 2
        o = g * 1536 + kc * 256 + h2 * 128
        return wqk_sb[:, o:o + 128]

    # ---- phase A: per-head qk projection [q64|k64 rows, tok] ----
    def phase_A_half(h, pqk, half):
        sl = slice(half * 512, half * 512 + 512)
        for kc in range(6):
            nc.tensor.matmul(pqk[:, sl], wqk_ap(kc, h),
                             xT_sb[:, kc * N + half * 512:
                                   kc * N + half * 512 + 512],
                             start=(kc == 0), stop=(kc == 5))

    def phase_A_mm(h):
        pqk = ps_a.tile([128, N], F32, tag="a", name="pqk")
        phase_A_half(h, pqk, 0)
        phase_A_half(h, pqk, 1)
        return pqk

    def phase_A_qcopy(h, pqk, half):
        lc = lhsT_c[h % 3]
        sl = slice(half * 512, half * 512 + 512)
        if with_qk_bias:
            nc.scalar.activation(lc[0:64, sl], pqk[0:64, sl], IDENTF,
                                 bias=tbl_sb[0:64, 2176 + 2 * h:2177 + 2 * h])
        else:
            nc.vector.tensor_copy(lc[0:64, sl], pqk[0:64, sl])

    def phase_A_kcopy(h, pqk, half):
        rc = rhs_c[h % 3]
        sl = slice(half * 512, half * 512 + 512)
        if with_qk_bias:
            nc.scalar.activation(rc[0:64, sl], pqk[64:128, sl], IDENTF,
                                 bias=tbl_sb[0:64, 2177 + 2 * h:2178 + 2 * h])
        else:
            nc.vector.tensor_copy(rc[0:64, sl], pqk[64:128, sl])

    def phase_A_copies(h, pqk):
        phase_A_qcopy(h, pqk, 0)
        phase_A_qcopy(h, pqk, 1)
        phase_A_kcopy(h, pqk, 0)
        phase_A_kcopy(h, pqk, 1)

    # ---- phase C: rel-pos tables -> bias rows of lhsT_c ----
    # prh/prw share one [64, N] psum tile (partitions 0:32 / 32:64) from the
    # ps_a pool, sequenced after pqk's drain, so this phase never waits on
    # the exp-paced S-pool slots.
    def phase_C_mm(h, pool=None, tag="a"):
        lc = lhsT_c[h % 3]
        qT = lc[0:64, :]
        prhw = (pool or ps_a).tile([64, N], F32, tag=tag, name="prhw")
        prh = prhw[0:32, :]
        prw = prhw[32:64, :]
        for qh in range(32):
            sl = slice(qh * 32, qh * 32 + 32)
            nc.tensor.matmul(prh[:, sl], rhT[:, sl], qT[:, sl],
                             start=True, stop=True)
        qT3 = qT.rearrange("p (a b) -> p b a", b=32)  # [64, qw, qh]
        for qw in range(32):
            sl = slice(qw * 32, qw * 32 + 32)
            nc.tensor.matmul(prw[:, sl], rwT[:, sl], qT3[:, qw, :],
                             start=True, stop=True)
        return prhw

    def phase_C_copies(h, prhw, act=False):
        lc = lhsT_c[h % 3]
        prh = prhw[0:32, :]
        prw_v = prhw[32:64, :].rearrange("p (a b) -> p b a", b=32)
        if act:
            nc.scalar.copy(lc[64:96, 0:512], prh[:, 0:512])
            nc.scalar.copy(lc[96:128, 0:512], prw_v[:, 0:16, :])
        else:
            nc.vector.tensor_copy(lc[64:96, 0:512], prh[:, 0:512])
            nc.vector.tensor_copy(lc[96:128, 0:512], prw_v[:, 0:16, :])
        nc.vector.tensor_copy(lc[64:96, 512:1024], prh[:, 512:1024])
        nc.vector.tensor_copy(lc[96:128, 512:1024], prw_v[:, 16:32, :])

    # ---- phase B: V projection (token-major, ones column memset) ----
    def phase_B(m):
        pv = ps_av.tile([128, 6 * 64], F32, tag="av", name="pv")
        for kc in range(6):
            nc.tensor.matmul(pv[:], xT_sb[:, kc * N + m * 128:
                                          kc * N + m * 128 + 128],
                             wv_sb[:, kc * 384:kc * 384 + 384],
                             start=(kc == 0), stop=(kc == 5))
        dst = v_sb[m][:].rearrange("p (h c) -> p h c", c=65)
        if m % 2 == 0:
            nc.vector.tensor_copy(dst[:, :, 0:64],
                                  pv[:].rearrange("p (h c) -> p h c", c=64))
        else:
            nc.scalar.copy(dst[:, :, 0:64],
                           pv[:].rearrange("p (h c) -> p h c", c=64))
        nc.vector.memset(dst[:, :, 64], 1.0)

    # ---- phase D building blocks ----
    # The PE executes strictly in program order, so the emission order below
    # IS the schedule: the exp stream on Activation is the pacer (8x1038ns
    # per head) and every PE instruction is placed so its deps are satisfied
    # when its turn comes. Fillers: pqk two heads ahead, rel tables one head
    # ahead, previous head's AV(7)/normalize/transpose at the ladder top.
    pavs = {}
    norms = {}

    def S_unit(h, kb):
        lc, rc = lhsT_c[h % 3], rhs_c[h % 3]
        if kb == 7 and h >= 4:  # ps_a is idle in the last two ladders
            ps = ps_a.tile([128, N], F32, tag="a", name="s_ps")
        else:
            ps = ps_s.tile([128, N], F32, tag="s", name="s_ps")
        for half in range(2):
            sl = slice(half * 512, half * 512 + 512)
            nc.tensor.matmul(ps[:, sl], rc[:, kb * 128:kb * 128 + 128],
                             lc[:, sl], start=True, stop=True)
        nc.scalar.activation(attnT[kb][:], ps[:], EXPF, scale=0.125)

    def AV_unit(h, kb):
        if kb == 0:
            # explicit zero + accumulate-only matmuls: interleaved start=True
            # groups within one PSUM bank corrupt earlier regions on HW
            pavs[h] = [
                ps_av.tile([128, 260], F32, tag="av", name="pav_a"),
                ps_av.tile([128, 260], F32, tag="av", name="pav_b")]
            for pv_ in pavs[h]:
                nc.tensor.matmul(pv_[:], ident, zeros[:], start=True,
                                 stop=False, skip_group_check=True)
        for qc in range(8):
            r = (qc % 4) * 65
            nc.tensor.matmul(pavs[h][qc // 4][:, r:r + 65],
                             attnT[kb][:, qc * 128:qc * 128 + 128],
                             v_sb[kb][:, h * 65:h * 65 + 65],
                             start=False, stop=(kb == 7),
                             skip_group_check=True)

    COPYF = mybir.ActivationFunctionType.Copy

    def norm_gq(h, gq, act=False):
        recip = small.tile([128, 4], F32, tag="recip", name="recip")
        norm = small.tile([128, 256], F16, tag="norm", name="norm")
        norms[(h, gq)] = norm
        dens = pavs[h][gq][:].rearrange("p (a b) -> p a b", b=65)[:, :, 64]
        nc.vector.reciprocal_approx_fast(out=recip[:], in_=dens)
        for qc4 in range(4):
            r = qc4 * 65
            dst = norm[:, qc4 * 64:qc4 * 64 + 64]
            if act and qc4 % 2 == 0:
                nc.scalar.activation(dst, pavs[h][gq][:, r:r + 64], COPYF,
                                     scale=recip[:, qc4:qc4 + 1])
            else:
                nc.vector.tensor_scalar_mul(
                    dst, pavs[h][gq][:, r:r + 64], recip[:, qc4:qc4 + 1])

    def transp_gq(h, gq, act=False):  # 4 PE transposes + copy out
        norm = norms.pop((h, gq))
        pt = ps_av.tile([64, 512], F16, tag="av", name="pt")
        for qc4 in range(4):
            nc.tensor.transpose(pt[:, qc4 * 128:qc4 * 128 + 128],
                                norm[:, qc4 * 64:qc4 * 64 + 64], ident)
        dst = proj_lhsT[h // 2][(h % 2) * 64:(h % 2) * 64 + 64,
                                gq * 512:gq * 512 + 512]
        if act:
            nc.scalar.copy(dst, pt[:])
        else:
            nc.vector.tensor_copy(dst, pt[:])

    # ---- main pipeline ----
    # The scheduler follows emission order closely; this ordering interleaves
    # next-head qk projection (chopped into token halves with immediate
    # half-drains, to keep the single ps_a slot chain short) and rel tables
    # into the ladder's early windows, leaving the exp(6)/exp(7) windows for
    # the AV tail + normalize/transpose of the current head.
    pqks = {}

    def ladder(h):
        nxt, nxt2 = h + 1 < HPC, h + 2 < HPC
        last = h == HPC - 1
        S_unit(h, 0)
        S_unit(h, 1)
        S_unit(h, 2)
        if nxt2:
            pqks[h + 2] = ps_a.tile([128, N], F32, tag="a", name="pqk")
            phase_A_half(h + 2, pqks[h + 2], 0)
            phase_A_qcopy(h + 2, pqks[h + 2], 0)
        S_unit(h, 3)
        if nxt2:
            phase_A_half(h + 2, pqks[h + 2], 1)
            phase_A_qcopy(h + 2, pqks[h + 2], 1)
            phase_A_kcopy(h + 2, pqks[h + 2], 0)
            phase_A_kcopy(h + 2, pqks[h + 2], 1)
        S_unit(h, 4)
        AV_unit(h, 0)
        prhw = phase_C_mm(h + 2) if nxt2 else None
        S_unit(h, 5)
        AV_unit(h, 1)
        if nxt2:
            phase_C_copies(h + 2, prhw)
        S_unit(h, 6)
        AV_unit(h, 2)
        S_unit(h, 7)
        AV_unit(h, 3)
        AV_unit(h, 4)
        AV_unit(h, 5)
        AV_unit(h, 6)
        AV_unit(h, 7)
        norm_gq(h, 0)
        norm_gq(h, 1)
        transp_gq(h, 0)
        transp_gq(h, 1)

    # prologue: head 0 qk (two half-token tiles in the idle S pool, so each
    # half's copies drain without waiting for the other) + rel tables, also
    # via the S pool so nothing waits on the ps_a rotation.
    pqk0h = [ps_s.tile([128, 512], F32, tag="s", name=f"pqk0{x}")
             for x in range(2)]
    for half in range(2):
        for kc in range(6):
            o = kc * N + half * 512
            nc.tensor.matmul(pqk0h[half][:], wqk_ap(kc, 0),
                             xT_sb[:, o:o + 512],
                             start=(kc == 0), stop=(kc == 5))
    for half in range(2):
        sl = slice(half * 512, half * 512 + 512)
        if with_qk_bias:
            nc.scalar.activation(lhsT_c[0][0:64, sl], pqk0h[half][0:64, :],
                                 IDENTF, bias=tbl_sb[0:64, 2176:2177])
            nc.scalar.activation(rhs_c[0][0:64, sl], pqk0h[half][64:128, :],
                                 IDENTF, bias=tbl_sb[0:64, 2177:2178])
        else:
            nc.scalar.copy(lhsT_c[0][0:64, sl], pqk0h[half][0:64, :])
            if half == 0:
                nc.vector.tensor_copy(rhs_c[0][0:64, sl],
                                      pqk0h[half][64:128, :])
            else:
                nc.scalar.copy(rhs_c[0][0:64, sl], pqk0h[half][64:128, :])
    prhw0 = phase_C_mm(0, pool=ps_s, tag="s")
    phase_C_copies(0, prhw0, act=True)

    # ladder 0 (special): B interleaved, AV(0) deferred behind B; pqk(1),
    # C(1), pqk(2) as fillers
    S_unit(0, 0)
    phase_B(0)
    S_unit(0, 1)
    phase_B(1)
    pqks[1] = ps_a.tile([128, N], F32, tag="a", name="pqk")
    phase_A_half(1, pqks[1], 0)
    phase_A_qcopy(1, pqks[1], 0)
    S_unit(0, 2)
    phase_B(2)
    phase_A_half(1, pqks[1], 1)
    phase_A_qcopy(1, pqks[1], 1)
    phase_A_kcopy(1, pqks[1], 0)
    phase_A_kcopy(1, pqks[1], 1)
    S_unit(0, 3)
    phase_B(3)
    prhw1 = phase_C_mm(1)
    S_unit(0, 4)
    phase_B(4)
    phase_C_copies(1, prhw1)
    S_unit(0, 5)
    phase_B(5)
    pqks[2] = ps_a.tile([128, N], F32, tag="a", name="pqk")
    phase_A_half(2, pqks[2], 0)
    phase_A_qcopy(2, pqks[2], 0)
    phase_A_half(2, pqks[2], 1)
    phase_A_qcopy(2, pqks[2], 1)
    phase_A_kcopy(2, pqks[2], 0)
    phase_A_kcopy(2, pqks[2], 1)
    S_unit(0, 6)
    phase_B(6)
    prhw2 = phase_C_mm(2)
    phase_C_copies(2, prhw2)
    S_unit(0, 7)
    phase_B(7)
    for kb in range(8):
        AV_unit(0, kb)
    norm_gq(0, 0)
    norm_gq(0, 1)
    transp_gq(0, 0)
    transp_gq(0, 1)

    for h in range(1, HPC):
        ladder(h)

    # ---- phase E: projection ----
    def proj_final(m, pool, tag, split):
        if split:
            pa = pool.tile([128, 512], F32, tag="av", name="pp_a")
            pb = pool.tile([128, 256], F32, tag="av", name="pp_b")
            tiles = [(pa, 0, 0, 512), (pb, 0, 512, 256)]
        else:
            pp = pool.tile([128, N], F32, tag=tag, name="pp")
            tiles = [(pp, 0, 0, 512), (pp, 512, 512, 256)]
        for t in range(3):
            for tile_, o, n0, nw in tiles:
                nc.tensor.matmul(tile_[:, o:o + nw],
                                 proj_lhsT[t][:, m * 128:m * 128 + 128],
                                 wp_sb[:, t * 768 + n0:t * 768 + n0 + nw],
                                 start=(t == 0), stop=(t == 2))
        osb = outp.tile([128, DIM], F16, tag="osb", name="osb")
        for i, (tile_, o, n0, nw) in enumerate(tiles):
            if (m + i) % 2 == 0:
                nc.vector.tensor_copy(osb[:, n0:n0 + nw], tile_[:, o:o + nw])
            else:
                nc.scalar.copy(osb[:, n0:n0 + nw], tile_[:, o:o + nw])
        eng = nc.sync if m % 2 == 0 else nc.scalar
        eng.dma_start(out_d[m * 128:m * 128 + 128, :], osb[:])

    order = [(ps_a, "a", False), (ps_s, "s", False),
             (ps_s, "s", False), (ps_av, "av", True)]
    for m in range(8):
        pool, tag, split = order[m % 4]
        proj_final(m, pool, tag, split)


def _host_prep(x, qkv_w, qkv_b, proj_w, proj_b, rel_pos_h, rel_pos_w):
    idx_h = np.arange(H)[:, None] - np.arange(H)[None, :] + (H - 1)
    idx_w = np.arange(W)[:, None] - np.arange(W)[None, :] + (W - 1)
    rhT8 = (8.0 * rel_pos_h[idx_h]).transpose(2, 0, 1).reshape(HD, H * H)
    rwT8 = (8.0 * rel_pos_w[idx_w]).transpose(2, 0, 1).reshape(HD, W * W)
    kt = np.arange(N)
    ec = np.zeros((64, N), np.float32)
    ec[:32] = (np.arange(32)[:, None] == (kt // 32)[None, :])
    ec[32:] = (np.arange(32)[:, None] == (kt % 32)[None, :])

    in_maps = []
    for core in range(NCORES):
        b = core // 2
        h0 = (core % 2) * HPC
        xb = x[b].reshape(N, DIM)
        xT_d = np.ascontiguousarray(
            xb.T.reshape(6, 128, N).transpose(1, 0, 2).reshape(128, 6 * N))

        wqk_t = np.zeros((DIM, 6 * 128), np.float32)  # [in, h*128 + (q|k)]
        wv_t = np.zeros((DIM, 6 * 64), np.float32)
        qb = np.zeros((64, 12), np.float32)
        for h in range(HPC):
            g = h0 + h
            wqk_t[:, h * 128:h * 128 + 64] = qkv_w[g * HD:(g + 1) * HD].T
            wqk_t[:, h * 128 + 64:h * 128 + 128] = \
                qkv_w[DIM + g * HD:DIM + (g + 1) * HD].T
            wv_t[:, h * 64:(h + 1) * 64] = \
                qkv_w[2 * DIM + g * HD:2 * DIM + (g + 1) * HD].T
            qb[:, 2 * h] = qkv_b[g * HD:(g + 1) * HD]
            qb[:, 2 * h + 1] = qkv_b[DIM + g * HD:DIM + (g + 1) * HD]
        # wqk packed: [p, g2*1536 + kc*256 + h2*128 + c]
        wqk_d = np.ascontiguousarray(
            wqk_t.reshape(6, 128, 3, 256).transpose(1, 2, 0, 3)
            .reshape(128, 4608))
        wv_d = np.ascontiguousarray(
            wv_t.reshape(6, 128, 384).transpose(1, 0, 2).reshape(128, 2304))

        wpm = np.zeros((HPC * HD, DIM), np.float32)
        for h in range(HPC):
            g = h0 + h
            wpm[h * HD:(h + 1) * HD, :] = proj_w[:, g * HD:(g + 1) * HD].T
        wp_d = np.ascontiguousarray(
            wpm.reshape(3, 128, 768).transpose(1, 0, 2).reshape(128, 2304))

        tbl_d = np.zeros((128, 2304), np.float32)
        tbl_d[0:64, 0:1024] = rhT8
        tbl_d[64:128, 0:1024] = rwT8
        tbl_d[0:64, 1024:2048] = ec
        tbl_d[:, 2048:2176] = np.eye(128)
        tbl_d[0:64, 2176:2188] = qb

        in_maps.append({
            "xT": xT_d.astype(np.float16),
            "wqk": wqk_d.astype(np.float16),
            "wv": wv_d.astype(np.float16),
            "wp": wp_d.astype(np.float16),
            "tbl": tbl_d.astype(np.float16),
        })
    return in_maps


def kernel(x, qkv_w, qkv_b, proj_w, proj_b, rel_pos_h, rel_pos_w, _trace=False):
    x = np.asarray(x, np.float32)
    qkv_w = np.asarray(qkv_w, np.float32)
    qkv_b = np.asarray(qkv_b, np.float32)
    proj_w = np.asarray(proj_w, np.float32)
    proj_b = np.asarray(proj_b, np.float32)
    rel_pos_h = np.asarray(rel_pos_h, np.float32)
    rel_pos_w = np.asarray(rel_pos_w, np.float32)

    in_maps = _host_prep(x, qkv_w, qkv_b, proj_w, proj_b,
                         rel_pos_h, rel_pos_w)
    with_qk_bias = bool(np.any(qkv_b[:2 * DIM]))
    key = ("nc", with_qk_bias)
    if key not in _cache:
        _cache[key] = build_program(with_qk_bias)
    nc = _cache[key]
    res = run_bass_kernel_spmd(nc, in_maps, core_ids=list(range(NCORES)),
                               trace=_trace)
    parts = [np.asarray(r["out_part"], np.float32) for r in res.results]
    pb_eff = proj_b + proj_w @ qkv_b[2 * DIM:]
    out = np.zeros((B, N, DIM), np.float32)
    for b in range(B):
        out[b] = parts[2 * b] + parts[2 * b + 1] + pb_eff
    if _trace:
        kernel.last_results = res
    return out.reshape(B, H, W, DIM)


# revision 67
# speedup vs baseline: 1.0079x; 1.0079x over previous
"""Trainium2 Bass kernel for decomposed-rel-pos attention (B=4, H=W=32, DIM=768, HEADS=12).

Sharding: 48 (batch, head) pairs -> 8 cores x 6 heads (core c: batch c//2,
heads (c%2)*6 .. +6). Each core computes qkv for its heads, attention with the
decomposed rel-pos bias folded into the S matmul as extra contraction rows
(0/1 expander matrices), softmax without max-subtraction, row-sums via a
ones-column appended to V, and a partial head-projection. Host sums the two
half-head partials per batch and adds an effective proj bias (which also
absorbs the v-bias exactly, since softmax rows sum to 1).

Numerics: all device tensors are fp16 (PE runs fp16 at 1 cycle/row with no
small-N penalty; PSUM accumulation is fp32). The softmax scale (1/8) is folded
into the exp activation's scale operand, with the rel-pos tables pre-scaled by
8 on host so the bias term comes out unscaled. End-to-end rel err vs the fp32
jax reference: ~1e-3 (tolerance 2e-2).

Layout/throughput notes (~102us/core cost-model estimate, vs 133us for the
previous f32r feature-major version):
- AV runs token-major (out [q,65] tiles, ap=65 per matmul; stationary = attnT
  slices): 4160 cycles/head + 1024 for the PE transpose back to feature-major,
  vs 8192 feature-major. Normalization becomes a per-partition scalar multiply
  (TensorScalarPtr) with one batched reciprocal per 4-chunk group.
- The pav accumulators are zeroed by a PE matmul against a zeros tile and all
  AV matmuls accumulate (start=False): interleaved start=True accumulation
  groups within one PSUM bank corrupt earlier regions on hardware.
- GPSIMD cannot access PSUM, so every PSUM-reading copy is on DVE (steady
  state) or Act (prologue/tail, where exp is not running).
- Inputs are host-packed into SBUF-layout [128, X] dram tensors so each loads
  with one large-descriptor DMA (10 input DMAs; HWDGE serialization is ~630ns
  per dma_start); xT is split in 3 so the qk projection starts early.
- Software pipeline: ladder(h) runs head h's S/exp stream (the pacer: 8 x
  ~1040ns exps on Act) and interleaves, as idle fillers: qk projection and
  rel tables for head h+2 (two ladders of slack on the single ps_a slot
  chain), the previous head's AV tail + normalize + transpose, and this
  head's AV units trailing the exps by 3.
- PSUM budget is exactly 8 banks: S pool 2x[128,1024], AV pool 2x[128,512]
  (pav accumulators then transpose targets), qk/rel-table pool 1x[128,1024].
"""
from contextlib import ExitStack

import numpy as np

import concourse.bass as bass
import concourse.bacc as bacc
import concourse.mybir as mybir
import concourse.tile as tile
from concourse.bass_utils import run_bass_kernel_spmd

B, H, W, DIM, HEADS = 4, 32, 32, 768, 12
HD = DIM // HEADS  # 64
N = H * W  # 1024
HPC = HEADS // 2  # heads per core = 6
NCORES = 8
F32 = mybir.dt.float32
F16 = mybir.dt.float16
EXPF = mybir.ActivationFunctionType.Exp
IDENTF = mybir.ActivationFunctionType.Identity

_cache = {}


def build_program(with_qk_bias=False):
    nc = bacc.Bacc("TRN2", target_bir_lowering=False, debug=False,
                   enable_asserts=False, num_devices=NCORES)
    xT = nc.dram_tensor("xT", [128, 6 * N], F16, kind="ExternalInput")
    wqk = nc.dram_tensor("wqk", [128, 6 * 768], F16, kind="ExternalInput")
    wv = nc.dram_tensor("wv", [128, 6 * 384], F16, kind="ExternalInput")
    wp = nc.dram_tensor("wp", [128, 3 * 768], F16, kind="ExternalInput")
    tbl = nc.dram_tensor("tbl", [128, 2304], F16, kind="ExternalInput")
    out_d = nc.dram_tensor("out_part", [N, DIM], F16, kind="ExternalOutput")

    with ExitStack() as ctx:
        tc = ctx.enter_context(tile.TileContext(nc))
        _body(nc, tc, ctx, xT, wqk, wv, wp, tbl, out_d, with_qk_bias)
    nc.compile()
    return nc


def _body(nc, tc, ctx, xT, wqk, wv, wp, tbl, out_d, with_qk_bias):
    persist = ctx.enter_context(tc.tile_pool(name="persist", bufs=1))
    attn_pool = ctx.enter_context(tc.tile_pool(name="attn", bufs=1))
    small = ctx.enter_context(tc.tile_pool(name="small", bufs=2))
    outp = ctx.enter_context(tc.tile_pool(name="outp", bufs=6))
    ps_s = ctx.enter_context(tc.tile_pool(name="ps_s", bufs=2, space="PSUM"))
    ps_av = ctx.enter_context(tc.tile_pool(name="ps_av", bufs=2, space="PSUM"))
    ps_a = ctx.enter_context(tc.tile_pool(name="ps_a", bufs=1, space="PSUM"))

    # ---- persistent SBUF tiles ----
    xT_sb = persist.tile([128, 6 * N], F16, tag="xt", name="xt")
    wqk_sb = persist.tile([128, 6 * 768], F16, tag="wqk", name="wqk")
    wv_sb = persist.tile([128, 6 * 384], F16, tag="wv", name="wv")
    wp_sb = persist.tile([128, 3 * 768], F16, tag="wp", name="wp")
    tbl_sb = persist.tile([128, 2304], F16, tag="tbl", name="tbl")
    rhT = tbl_sb[0:64, 0:1024]
    rwT = persist.tile([64, 1024], F16, tag="rwT", name="rwT")
    ecomb = tbl_sb[0:64, 1024:2048]
    ident = tbl_sb[:, 2048:2176]
    v_sb = [persist.tile([128, HPC * 65], F16, tag=f"v{m}", name=f"v{m}")
            for m in range(8)]
    proj_lhsT = [persist.tile([128, N], F16, tag=f"pl{t}", name=f"pl{t}")
                 for t in range(3)]
    lhsT_c = [persist.tile([128, N], F16, tag=f"lhs{i}", name=f"lhs{i}")
              for i in range(3)]
    rhs_c = [persist.tile([128, N], F16, tag=f"rhs{i}", name=f"rhs{i}")
             for i in range(3)]
    attnT = [attn_pool.tile([128, N], F16, tag=f"attnT{kb}", name=f"attnT{kb}")
             for kb in range(8)]
    zeros = persist.tile([128, 260], F16, tag="zeros", name="zeros")
    nc.vector.memset(zeros[:], 0.0)

    # ---- input DMAs: two queues, interleaved; each is one large transfer ----
    # wqk is packed by head-pair group g: col g*1536 + kc*256 + (h%2)*128 + c,
    # so group g arrives early enough to gate only heads 2g, 2g+1.
    nc.sync.dma_start(xT_sb[:, 0:N], xT[:, 0:N])
    nc.sync.dma_start(xT_sb[:, N:3 * N], xT[:, N:3 * N])
    nc.scalar.dma_start(wqk_sb[:, 0:1536], wqk[:, 0:1536])
    nc.scalar.dma_start(xT_sb[:, 3 * N:6 * N], xT[:, 3 * N:6 * N])
    nc.scalar.dma_start(tbl_sb[:, 0:2048], tbl[:, 0:2048])
    nc.scalar.dma_start(tbl_sb[:, 2048:2304], tbl[:, 2048:2304])
    nc.scalar.dma_start(wv_sb[:], wv[:])
    nc.scalar.dma_start(wqk_sb[:, 1536:3072], wqk[:, 1536:3072])
    nc.scalar.dma_start(wqk_sb[:, 3072:4608], wqk[:, 3072:4608])
    nc.scalar.dma_start(wp_sb[:], wp[:])

    # ecomb rows into both rhs buffers once (rows 64:128 never rewritten);
    # rwT to a partition-0 tile (PE matmul needs matching base partitions)
    nc.vector.tensor_copy(rwT[:], tbl_sb[64:128, 0:1024])
    nc.vector.tensor_copy(rhs_c[0][64:128, :], ecomb)
    nc.vector.tensor_copy(rhs_c[1][64:128, :], ecomb)
    nc.vector.tensor_copy(rhs_c[2][64:128, :], ecomb)

    def wqk_ap(kc, h):
        g, h2 = h // 2, h % 2
        o = g * 1536 + kc * 256 + h2 * 128
        return wqk_sb[:, o:o + 128]

    # ---- phase A: per-head qk projection [q64|k64 rows, tok] ----
    def phase_A_half(h, pqk, half):
        sl = slice(half * 512, half * 512 + 512)
        for kc in range(6):
            nc.tensor.matmul(pqk[:, sl], wqk_ap(kc, h),
                             xT_sb[:, kc * N + half * 512:
                                   kc * N + half * 512 + 512],
                             start=(kc == 0), stop=(kc == 5))

    def phase_A_mm(h):
        pqk = ps_a.tile([128, N], F32, tag="a", name="pqk")
        phase_A_half(h, pqk, 0)
        phase_A_half(h, pqk, 1)
        return pqk

    def phase_A_qcopy(h, pqk, half):
        lc = lhsT_c[h % 3]
        sl = slice(half * 512, half * 512 + 512)
        if with_qk_bias:
            nc.scalar.activation(lc[0:64, sl], pqk[0:64, sl], IDENTF,
                                 bias=tbl_sb[0:64, 2176 + 2 * h:2177 + 2 * h])
        else:
            nc.vector.tensor_copy(lc[0:64, sl], pqk[0:64, sl])

    def phase_A_kcopy(h, pqk, half):
        rc = rhs_c[h % 3]
        sl = slice(half * 512, half * 512 + 512)
        if with_qk_bias:
            nc.scalar.activation(rc[0:64, sl], pqk[64:128, sl], IDENTF,
                                 bias=tbl_sb[0:64, 2177 + 2 * h:2178 + 2 * h])
        else:
            nc.vector.tensor_copy(rc[0:64, sl], pqk[64:128, sl])

    def phase_A_copies(h, pqk):
        phase_A_qcopy(h, pqk, 0)
        phase_A_qcopy(h, pqk, 1)
        phase_A_kcopy(h, pqk, 0)
        phase_A_kcopy(h, pqk, 1)

    # ---- phase C: rel-pos tables -> bias rows of lhsT_c ----
    # prh/prw share one [64, N] psum tile (partitions 0:32 / 32:64) from the
    # ps_a pool, sequenced after pqk's drain, so this phase never waits on
    # the exp-paced S-pool slots.
    def phase_C_mm(h, pool=None, tag="a"):
        lc = lhsT_c[h % 3]
        qT = lc[0:64, :]
        prhw = (pool or ps_a).tile([64, N], F32, tag=tag, name="prhw")
        prh = prhw[0:32, :]
        prw = prhw[32:64, :]
        for qh in range(32):
            sl = slice(qh * 32, qh * 32 + 32)
            nc.tensor.matmul(prh[:, sl], rhT[:, sl], qT[:, sl],
                             start=True, stop=True)
        qT3 = qT.rearrange("p (a b) -> p b a", b=32)  # [64, qw, qh]
        for qw in range(32):
            sl = slice(qw * 32, qw * 32 + 32)
            nc.tensor.matmul(prw[:, sl], rwT[:, sl], qT3[:, qw, :],
                             start=True, stop=True)
        return prhw

    def phase_C_copies(h, prhw, act=False):
        lc = lhsT_c[h % 3]
        prh = prhw[0:32, :]
        prw_v = prhw[32:64, :].rearrange("p (a b) -> p b a", b=32)
        if act:
            nc.scalar.copy(lc[64:96, 0:512], prh[:, 0:512])
            nc.scalar.copy(lc[96:128, 0:512], prw_v[:, 0:16, :])
        else:
            nc.vector.tensor_copy(lc[64:96, 0:512], prh[:, 0:512])
            nc.vector.tensor_copy(lc[96:128, 0:512], prw_v[:, 0:16, :])
        nc.vector.tensor_copy(lc[64:96, 512:1024], prh[:, 512:1024])
        nc.vector.tensor_copy(lc[96:128, 512:1024], prw_v[:, 16:32, :])

    # ---- phase B: V projection (token-major, ones column memset) ----
    def phase_B(m):
        pv = ps_av.tile([128, 6 * 64], F32, tag="av", name="pv")
        for kc in range(6):
            nc.tensor.matmul(pv[:], xT_sb[:, kc * N + m * 128:
                                          kc * N + m * 128 + 128],
                             wv_sb[:, kc * 384:kc * 384 + 384],
                             start=(kc == 0), stop=(kc == 5))
        dst = v_sb[m][:].rearrange("p (h c) -> p h c", c=65)
        if m % 2 == 0:
            nc.vector.tensor_copy(dst[:, :, 0:64],
                                  pv[:].rearrange("p (h c) -> p h c", c=64))
        else:
            nc.scalar.copy(dst[:, :, 0:64],
                           pv[:].rearrange("p (h c) -> p h c", c=64))
        nc.vector.memset(dst[:, :, 64], 1.0)

    # ---- phase D building blocks ----
    # The PE executes strictly in program order, so the emission order below
    # IS the schedule: the exp stream on Activation is the pacer (8x1038ns
    # per head) and every PE instruction is placed so its deps are satisfied
    # when its turn comes. Fillers: pqk two heads ahead, rel tables one head
    # ahead, previous head's AV(7)/normalize/transpose at the ladder top.
    pavs = {}
    norms = {}

    def S_unit(h, kb):
        lc, rc = lhsT_c[h % 3], rhs_c[h % 3]
        if kb == 7 and h >= 4:  # ps_a is idle in the last two ladders
            ps = ps_a.tile([128, N], F32, tag="a", name="s_ps")
        else:
            ps = ps_s.tile([128, N], F32, tag="s", name="s_ps")
        for half in range(2):
            sl = slice(half * 512, half * 512 + 512)
            nc.tensor.matmul(ps[:, sl], rc[:, kb * 128:kb * 128 + 128],
                             lc[:, sl], start=True, stop=True)
        if kb == 7 and h == HPC - 1:
            # split the very last exp so the AV stop -> normalize ->
            # transpose chain (which gates the projection) starts earlier
            nc.scalar.activation(attnT[kb][:, 0:512], ps[:, 0:512],
                                 EXPF, scale=0.125)
            nc.scalar.activation(attnT[kb][:, 512:1024], ps[:, 512:1024],
                                 EXPF, scale=0.125)
        else:
            nc.scalar.activation(attnT[kb][:], ps[:], EXPF, scale=0.125)

    def AV_unit(h, kb):
        if kb == 0:
            # explicit zero + accumulate-only matmuls: interleaved start=True
            # groups within one PSUM bank corrupt earlier regions on HW
            pavs[h] = [
                ps_av.tile([128, 260], F32, tag="av", name="pav_a"),
                ps_av.tile([128, 260], F32, tag="av", name="pav_b")]
            for pv_ in pavs[h]:
                nc.tensor.matmul(pv_[:], ident, zeros[:], start=True,
                                 stop=False, skip_group_check=True)
        for qc in range(8):
            r = (qc % 4) * 65
            nc.tensor.matmul(pavs[h][qc // 4][:, r:r + 65],
                             attnT[kb][:, qc * 128:qc * 128 + 128],
                             v_sb[kb][:, h * 65:h * 65 + 65],
                             start=False, stop=(kb == 7),
                             skip_group_check=True)

    COPYF = mybir.ActivationFunctionType.Copy

    def norm_gq(h, gq, act=False):
        recip = small.tile([128, 4], F32, tag="recip", name="recip")
        norm = small.tile([128, 256], F16, tag="norm", name="norm")
        norms[(h, gq)] = norm
        dens = pavs[h][gq][:].rearrange("p (a b) -> p a b", b=65)[:, :, 64]
        nc.vector.reciprocal_approx_fast(out=recip[:], in_=dens)
        for qc4 in range(4):
            r = qc4 * 65
            dst = norm[:, qc4 * 64:qc4 * 64 + 64]
            if act and qc4 % 2 == 0:
                nc.scalar.activation(dst, pavs[h][gq][:, r:r + 64], COPYF,
                                     scale=recip[:, qc4:qc4 + 1])
            else:
                nc.vector.tensor_scalar_mul(
                    dst, pavs[h][gq][:, r:r + 64], recip[:, qc4:qc4 + 1])

    def transp_gq(h, gq, act=False):  # 4 PE transposes + copy out
        norm = norms.pop((h, gq))
        pt = ps_av.tile([64, 512], F16, tag="av", name="pt")
        for qc4 in range(4):
            nc.tensor.transpose(pt[:, qc4 * 128:qc4 * 128 + 128],
                                norm[:, qc4 * 64:qc4 * 64 + 64], ident)
        dst = proj_lhsT[h // 2][(h % 2) * 64:(h % 2) * 64 + 64,
                                gq * 512:gq * 512 + 512]
        if act:
            nc.scalar.copy(dst, pt[:])
        else:
            nc.vector.tensor_copy(dst, pt[:])

    # ---- main pipeline ----
    # The scheduler follows emission order closely; this ordering interleaves
    # next-head qk projection (chopped into token halves with immediate
    # half-drains, to keep the single ps_a slot chain short) and rel tables
    # into the ladder's early windows, leaving the exp(6)/exp(7) windows for
    # the AV tail + normalize/transpose of the current head.
    pqks = {}

    def ladder(h):
        nxt, nxt2 = h + 1 < HPC, h + 2 < HPC
        last = h == HPC - 1
        S_unit(h, 0)
        S_unit(h, 1)
        S_unit(h, 2)
        if nxt2:
            pqks[h + 2] = ps_a.tile([128, N], F32, tag="a", name="pqk")
            phase_A_half(h + 2, pqks[h + 2], 0)
            phase_A_qcopy(h + 2, pqks[h + 2], 0)
        S_unit(h, 3)
        if nxt2:
            phase_A_half(h + 2, pqks[h + 2], 1)
            phase_A_qcopy(h + 2, pqks[h + 2], 1)
            phase_A_kcopy(h + 2, pqks[h + 2], 0)
            phase_A_kcopy(h + 2, pqks[h + 2], 1)
        S_unit(h, 4)
        AV_unit(h, 0)
        prhw = phase_C_mm(h + 2) if nxt2 else None
        S_unit(h, 5)
        AV_unit(h, 1)
        if nxt2:
            phase_C_copies(h + 2, prhw)
        S_unit(h, 6)
        AV_unit(h, 2)
        S_unit(h, 7)
        AV_unit(h, 3)
        AV_unit(h, 4)
        AV_unit(h, 5)
        AV_unit(h, 6)
        AV_unit(h, 7)
        norm_gq(h, 0)
        norm_gq(h, 1)
        transp_gq(h, 0)
        transp_gq(h, 1)

    # prologue: head 0 qk (two half-token tiles in the idle S pool, so each
    # half's copies drain without waiting for the other) + rel tables, also
    # via the S pool so nothing waits on the ps_a rotation.
    pqk0h = [ps_s.tile([128, 512], F32, tag="s", name=f"pqk0{x}")
             for x in range(2)]
    for half in range(2):
        for kc in range(6):
            o = kc * N + half * 512
            nc.tensor.matmul(pqk0h[half][:], wqk_ap(kc, 0),
                             xT_sb[:, o:o + 512],
                             start=(kc == 0), stop=(kc == 5))
    for half in range(2):
        sl = slice(half * 512, half * 512 + 512)
        if with_qk_bias:
            nc.scalar.activation(lhsT_c[0][0:64, sl], pqk0h[half][0:64, :],
                                 IDENTF, bias=tbl_sb[0:64, 2176:2177])
            nc.scalar.activation(rhs_c[0][0:64, sl], pqk0h[half][64:128, :],
                                 IDENTF, bias=tbl_sb[0:64, 2177:2178])
        else:
            nc.scalar.copy(lhsT_c[0][0:64, sl], pqk0h[half][0:64, :])
            if half == 0:
                nc.vector.tensor_copy(rhs_c[0][0:64, sl],
                                      pqk0h[half][64:128, :])
            else:
                nc.scalar.copy(rhs_c[0][0:64, sl], pqk0h[half][64:128, :])
    prhw0 = phase_C_mm(0, pool=ps_s, tag="s")
    phase_C_copies(0, prhw0, act=True)

    # ladder 0 (special): B interleaved, AV(0) deferred behind B; pqk(1),
    # C(1), pqk(2) as fillers
    S_unit(0, 0)
    phase_B(0)
    S_unit(0, 1)
    phase_B(1)
    pqks[1] = ps_a.tile([128, N], F32, tag="a", name="pqk")
    phase_A_half(1, pqks[1], 0)
    phase_A_qcopy(1, pqks[1], 0)
    S_unit(0, 2)
    phase_B(2)
    phase_A_half(1, pqks[1], 1)
    phase_A_qcopy(1, pqks[1], 1)
    phase_A_kcopy(1, pqks[1], 0)
    phase_A_kcopy(1, pqks[1], 1)
    S_unit(0, 3)
    phase_B(3)
    prhw1 = phase_C_mm(1)
    S_unit(0, 4)
    phase_B(4)
    phase_C_copies(1, prhw1)
    S_unit(0, 5)
    phase_B(5)
    pqks[2] = ps_a.tile([128, N], F32, tag="a", name="pqk")
    phase_A_half(2, pqks[2], 0)
    phase_A_qcopy(2, pqks[2], 0)
    phase_A_half(2, pqks[2], 1)
    phase_A_qcopy(2, pqks[2], 1)
    phase_A_kcopy(2, pqks[2], 0)
    phase_A_kcopy(2, pqks[2], 1)
    S_unit(0, 6)
    phase_B(6)
    prhw2 = phase_C_mm(2)
    phase_C_copies(2, prhw2)
    S_unit(0, 7)
    phase_B(7)
    for kb in range(8):
        AV_unit(0, kb)
    norm_gq(0, 0)
    norm_gq(0, 1)
    transp_gq(0, 0)
    transp_gq(0, 1)

    for h in range(1, HPC):
        ladder(h)

    # ---- phase E: projection ----
    def proj_final(m, pool, tag, split):
        if split:
            pa = pool.tile([128, 512], F32, tag="av", name="pp_a")
            pb = pool.tile([128, 256], F32, tag="av", name="pp_b")
            tiles = [(pa, 0, 0, 512), (pb, 0, 512, 256)]
        else:
            pp = pool.tile([128, N], F32, tag=tag, name="pp")
            tiles = [(pp, 0, 0, 512), (pp, 512, 512, 256)]
        for t in range(3):
            for tile_, o, n0, nw in tiles:
                nc.tensor.matmul(tile_[:, o:o + nw],
                                 proj_lhsT[t][:, m * 128:m * 128 + 128],
                                 wp_sb[:, t * 768 + n0:t * 768 + n0 + nw],
                                 start=(t == 0), stop=(t == 2))
        osb = outp.tile([128, DIM], F16, tag="osb", name="osb")
        for i, (tile_, o, n0, nw) in enumerate(tiles):
            if (m + i) % 2 == 0:
                nc.vector.tensor_copy(osb[:, n0:n0 + nw], tile_[:, o:o + nw])
            else:
                nc.scalar.copy(osb[:, n0:n0 + nw], tile_[:, o:o + nw])
        eng = nc.sync if m % 2 == 0 else nc.scalar
        eng.dma_start(out_d[m * 128:m * 128 + 128, :], osb[:])

    order = [(ps_a, "a", False), (ps_s, "s", False),
             (ps_s, "s", False), (ps_av, "av", True)]
    for m in range(8):
        pool, tag, split = order[m % 4]
        proj_final(m, pool, tag, split)


def _host_prep(x, qkv_w, qkv_b, proj_w, proj_b, rel_pos_h, rel_pos_w):
    idx_h = np.arange(H)[:, None] - np.arange(H)[None, :] + (H - 1)
    idx_w = np.arange(W)[:, None] - np.arange(W)[None, :] + (W - 1)
    rhT8 = (8.0 * rel_pos_h[idx_h]).transpose(2, 0, 1).reshape(HD, H * H)
    rwT8 = (8.0 * rel_pos_w[idx_w]).transpose(2, 0, 1).reshape(HD, W * W)
    kt = np.arange(N)
    ec = np.zeros((64, N), np.float32)
    ec[:32] = (np.arange(32)[:, None] == (kt // 32)[None, :])
    ec[32:] = (np.arange(32)[:, None] == (kt % 32)[None, :])

    in_maps = []
    for core in range(NCORES):
        b = core // 2
        h0 = (core % 2) * HPC
        xb = x[b].reshape(N, DIM)
        xT_d = np.ascontiguousarray(
            xb.T.reshape(6, 128, N).transpose(1, 0, 2).reshape(128, 6 * N))

        wqk_t = np.zeros((DIM, 6 * 128), np.float32)  # [in, h*128 + (q|k)]
        wv_t = np.zeros((DIM, 6 * 64), np.float32)
        qb = np.zeros((64, 12), np.float32)
        for h in range(HPC):
            g = h0 + h
            wqk_t[:, h * 128:h * 128 + 64] = qkv_w[g * HD:(g + 1) * HD].T
            wqk_t[:, h * 128 + 64:h * 128 + 128] = \
                qkv_w[DIM + g * HD:DIM + (g + 1) * HD].T
            wv_t[:, h * 64:(h + 1) * 64] = \
                qkv_w[2 * DIM + g * HD:2 * DIM + (g + 1) * HD].T
            qb[:, 2 * h] = qkv_b[g * HD:(g + 1) * HD]
            qb[:, 2 * h + 1] = qkv_b[DIM + g * HD:DIM + (g + 1) * HD]
        # wqk packed: [p, g2*1536 + kc*256 + h2*128 + c]
        wqk_d = np.ascontiguousarray(
            wqk_t.reshape(6, 128, 3, 256).transpose(1, 2, 0, 3)
            .reshape(128, 4608))
        wv_d = np.ascontiguousarray(
            wv_t.reshape(6, 128, 384).transpose(1, 0, 2).reshape(128, 2304))

        wpm = np.zeros((HPC * HD, DIM), np.float32)
        for h in range(HPC):
            g = h0 + h
            wpm[h * HD:(h + 1) * HD, :] = proj_w[:, g * HD:(g + 1) * HD].T
        wp_d = np.ascontiguousarray(
            wpm.reshape(3, 128, 768).transpose(1, 0, 2).reshape(128, 2304))

        tbl_d = np.zeros((128, 2304), np.float32)
        tbl_d[0:64, 0:1024] = rhT8
        tbl_d[64:128, 0:1024] = rwT8
        tbl_d[0:64, 1024:2048] = ec
        tbl_d[:, 2048:2176] = np.eye(128)
        tbl_d[0:64, 2176:2188] = qb

        in_maps.append({
            "xT": xT_d.astype(np.float16),
            "wqk": wqk_d.astype(np.float16),
            "wv": wv_d.astype(np.float16),
            "wp": wp_d.astype(np.float16),
            "tbl": tbl_d.astype(np.float16),
        })
    return in_maps


def kernel(x, qkv_w, qkv_b, proj_w, proj_b, rel_pos_h, rel_pos_w, _trace=False):
    x = np.asarray(x, np.float32)
    qkv_w = np.asarray(qkv_w, np.float32)
    qkv_b = np.asarray(qkv_b, np.float32)
    proj_w = np.asarray(proj_w, np.float32)
    proj_b = np.asarray(proj_b, np.float32)
    rel_pos_h = np.asarray(rel_pos_h, np.float32)
    rel_pos_w = np.asarray(rel_pos_w, np.float32)

    in_maps = _host_prep(x, qkv_w, qkv_b, proj_w, proj_b,
                         rel_pos_h, rel_pos_w)
    with_qk_bias = bool(np.any(qkv_b[:2 * DIM]))
    key = ("nc", with_qk_bias)
    if key not in _cache:
        _cache[key] = build_program(with_qk_bias)
    nc = _cache[key]
    res = run_bass_kernel_spmd(nc, in_maps, core_ids=list(range(NCORES)),
                               trace=_trace)
    parts = [np.asarray(r["out_part"], np.float32) for r in res.results]
    pb_eff = proj_b + proj_w @ qkv_b[2 * DIM:]
    out = np.zeros((B, N, DIM), np.float32)
    for b in range(B):
        out[b] = parts[2 * b] + parts[2 * b + 1] + pb_eff
    if _trace:
        kernel.last_results = res
    return out.reshape(B, H, W, DIM)


# revision 70
# speedup vs baseline: 1.0157x; 1.0078x over previous
"""Trainium2 Bass kernel for decomposed-rel-pos attention (B=4, H=W=32, DIM=768, HEADS=12).

Sharding: 48 (batch, head) pairs -> 8 cores x 6 heads (core c: batch c//2,
heads (c%2)*6 .. +6). Each core computes qkv for its heads, attention with the
decomposed rel-pos bias folded into the S matmul as extra contraction rows
(0/1 expander matrices), softmax without max-subtraction, row-sums via a
ones-column appended to V, and a partial head-projection. Host sums the two
half-head partials per batch and adds an effective proj bias (which also
absorbs the v-bias exactly, since softmax rows sum to 1).

Numerics: all device tensors are fp16 (PE runs fp16 at 1 cycle/row with no
small-N penalty; PSUM accumulation is fp32). The softmax scale (1/8) is folded
into the exp activation's scale operand, with the rel-pos tables pre-scaled by
8 on host so the bias term comes out unscaled. End-to-end rel err vs the fp32
jax reference: ~1e-3 (tolerance 2e-2).

Layout/throughput notes (~102us/core cost-model estimate, vs 133us for the
previous f32r feature-major version):
- AV runs token-major (out [q,65] tiles, ap=65 per matmul; stationary = attnT
  slices): 4160 cycles/head + 1024 for the PE transpose back to feature-major,
  vs 8192 feature-major. Normalization becomes a per-partition scalar multiply
  (TensorScalarPtr) with one batched reciprocal per 4-chunk group.
- The pav accumulators are zeroed by a PE matmul against a zeros tile and all
  AV matmuls accumulate (start=False): interleaved start=True accumulation
  groups within one PSUM bank corrupt earlier regions on hardware.
- GPSIMD cannot access PSUM, so every PSUM-reading copy is on DVE (steady
  state) or Act (prologue/tail, where exp is not running).
- Inputs are host-packed into SBUF-layout [128, X] dram tensors so each loads
  with one large-descriptor DMA (10 input DMAs; HWDGE serialization is ~630ns
  per dma_start); xT is split in 3 so the qk projection starts early.
- Software pipeline: ladder(h) runs head h's S/exp stream (the pacer: 8 x
  ~1040ns exps on Act) and interleaves, as idle fillers: qk projection and
  rel tables for head h+2 (two ladders of slack on the single ps_a slot
  chain), the previous head's AV tail + normalize + transpose, and this
  head's AV units trailing the exps by 3.
- PSUM budget is exactly 8 banks: S pool 2x[128,1024], AV pool 2x[128,512]
  (pav accumulators then transpose targets), qk/rel-table pool 1x[128,1024].
"""
from contextlib import ExitStack

import numpy as np

import concourse.bass as bass
import concourse.bacc as bacc
import concourse.mybir as mybir
import concourse.tile as tile
from concourse.bass_utils import run_bass_kernel_spmd

B, H, W, DIM, HEADS = 4, 32, 32, 768, 12
HD = DIM // HEADS  # 64
N = H * W  # 1024
HPC = HEADS // 2  # heads per core = 6
NCORES = 8
F32 = mybir.dt.float32
F16 = mybir.dt.float16
EXPF = mybir.ActivationFunctionType.Exp
IDENTF = mybir.ActivationFunctionType.Identity

_cache = {}


def build_program(with_qk_bias=False):
    nc = bacc.Bacc("TRN2", target_bir_lowering=False, debug=False,
                   enable_asserts=False, num_devices=NCORES)
    xT = nc.dram_tensor("xT", [128, 6 * N], F16, kind="ExternalInput")
    wqk = nc.dram_tensor("wqk", [128, 6 * 768], F16, kind="ExternalInput")
    wv = nc.dram_tensor("wv", [128, 6 * 384], F16, kind="ExternalInput")
    wp = nc.dram_tensor("wp", [128, 3 * 768], F16, kind="ExternalInput")
    tbl = nc.dram_tensor("tbl", [128, 2304], F16, kind="ExternalInput")
    out_d = nc.dram_tensor("out_part", [N, DIM], F16, kind="ExternalOutput")

    with ExitStack() as ctx:
        tc = ctx.enter_context(tile.TileContext(nc))
        _body(nc, tc, ctx, xT, wqk, wv, wp, tbl, out_d, with_qk_bias)
    nc.compile()
    return nc


def _body(nc, tc, ctx, xT, wqk, wv, wp, tbl, out_d, with_qk_bias):
    persist = ctx.enter_context(tc.tile_pool(name="persist", bufs=1))
    attn_pool = ctx.enter_context(tc.tile_pool(name="attn", bufs=1))
    small = ctx.enter_context(tc.tile_pool(name="small", bufs=2))
    outp = ctx.enter_context(tc.tile_pool(name="outp", bufs=6))
    ps_s = ctx.enter_context(tc.tile_pool(name="ps_s", bufs=2, space="PSUM"))
    ps_av = ctx.enter_context(tc.tile_pool(name="ps_av", bufs=2, space="PSUM"))
    ps_a = ctx.enter_context(tc.tile_pool(name="ps_a", bufs=1, space="PSUM"))

    # ---- persistent SBUF tiles ----
    xT_sb = persist.tile([128, 6 * N], F16, tag="xt", name="xt")
    wqk_sb = persist.tile([128, 6 * 768], F16, tag="wqk", name="wqk")
    wv_sb = persist.tile([128, 6 * 384], F16, tag="wv", name="wv")
    wp_sb = persist.tile([128, 3 * 768], F16, tag="wp", name="wp")
    tbl_sb = persist.tile([128, 2304], F16, tag="tbl", name="tbl")
    rhT = tbl_sb[0:64, 0:1024]
    rwT = persist.tile([64, 1024], F16, tag="rwT", name="rwT")
    ecomb = tbl_sb[0:64, 1024:2048]
    ident = tbl_sb[:, 2048:2176]
    v_sb = [persist.tile([128, HPC * 65], F16, tag=f"v{m}", name=f"v{m}")
            for m in range(8)]
    proj_lhsT = [persist.tile([128, N], F16, tag=f"pl{t}", name=f"pl{t}")
                 for t in range(3)]
    lhsT_c = [persist.tile([128, N], F16, tag=f"lhs{i}", name=f"lhs{i}")
              for i in range(3)]
    rhs_c = [persist.tile([128, N], F16, tag=f"rhs{i}", name=f"rhs{i}")
             for i in range(3)]
    attnT = [attn_pool.tile([128, N], F16, tag=f"attnT{kb}", name=f"attnT{kb}")
             for kb in range(8)]
    zeros = persist.tile([128, 260], F16, tag="zeros", name="zeros")
    nc.vector.memset(zeros[:], 0.0)

    # ---- input DMAs: two queues, interleaved; each is one large transfer ----
    # wqk is packed by head-pair group g: col g*1536 + kc*256 + (h%2)*128 + c,
    # so group g arrives early enough to gate only heads 2g, 2g+1.
    nc.sync.dma_start(xT_sb[:, 0:N], xT[:, 0:N])
    nc.sync.dma_start(xT_sb[:, N:3 * N], xT[:, N:3 * N])
    nc.scalar.dma_start(wqk_sb[:, 0:1536], wqk[:, 0:1536])
    nc.scalar.dma_start(xT_sb[:, 3 * N:6 * N], xT[:, 3 * N:6 * N])
    nc.scalar.dma_start(tbl_sb[:, 0:2048], tbl[:, 0:2048])
    nc.scalar.dma_start(tbl_sb[:, 2048:2304], tbl[:, 2048:2304])
    nc.scalar.dma_start(wv_sb[:], wv[:])
    nc.scalar.dma_start(wqk_sb[:, 1536:3072], wqk[:, 1536:3072])
    nc.scalar.dma_start(wqk_sb[:, 3072:4608], wqk[:, 3072:4608])
    nc.scalar.dma_start(wp_sb[:], wp[:])

    # ecomb rows into both rhs buffers once (rows 64:128 never rewritten);
    # rwT to a partition-0 tile (PE matmul needs matching base partitions)
    nc.vector.tensor_copy(rwT[:], tbl_sb[64:128, 0:1024])
    nc.vector.tensor_copy(rhs_c[0][64:128, :], ecomb)
    nc.vector.tensor_copy(rhs_c[1][64:128, :], ecomb)
    nc.vector.tensor_copy(rhs_c[2][64:128, :], ecomb)

    def wqk_ap(kc, h):
        g, h2 = h // 2, h % 2
        o = g * 1536 + kc * 256 + h2 * 128
        return wqk_sb[:, o:o + 128]

    # ---- phase A: per-head qk projection [q64|k64 rows, tok] ----
    def phase_A_half(h, pqk, half):
        sl = slice(half * 512, half * 512 + 512)
        for kc in range(6):
            nc.tensor.matmul(pqk[:, sl], wqk_ap(kc, h),
                             xT_sb[:, kc * N + half * 512:
                                   kc * N + half * 512 + 512],
                             start=(kc == 0), stop=(kc == 5))

    def phase_A_mm(h):
        pqk = ps_a.tile([128, N], F32, tag="a", name="pqk")
        phase_A_half(h, pqk, 0)
        phase_A_half(h, pqk, 1)
        return pqk

    def phase_A_qcopy(h, pqk, half):
        lc = lhsT_c[h % 3]
        sl = slice(half * 512, half * 512 + 512)
        if with_qk_bias:
            nc.scalar.activation(lc[0:64, sl], pqk[0:64, sl], IDENTF,
                                 bias=tbl_sb[0:64, 2176 + 2 * h:2177 + 2 * h])
        else:
            nc.vector.tensor_copy(lc[0:64, sl], pqk[0:64, sl])

    def phase_A_kcopy(h, pqk, half):
        rc = rhs_c[h % 3]
        sl = slice(half * 512, half * 512 + 512)
        if with_qk_bias:
            nc.scalar.activation(rc[0:64, sl], pqk[64:128, sl], IDENTF,
                                 bias=tbl_sb[0:64, 2177 + 2 * h:2178 + 2 * h])
        else:
            nc.vector.tensor_copy(rc[0:64, sl], pqk[64:128, sl])

    def phase_A_copies(h, pqk):
        phase_A_qcopy(h, pqk, 0)
        phase_A_qcopy(h, pqk, 1)
        phase_A_kcopy(h, pqk, 0)
        phase_A_kcopy(h, pqk, 1)

    # ---- phase C: rel-pos tables -> bias rows of lhsT_c ----
    # prh/prw share one [64, N] psum tile (partitions 0:32 / 32:64) from the
    # ps_a pool, sequenced after pqk's drain, so this phase never waits on
    # the exp-paced S-pool slots.
    def phase_C_mm(h, pool=None, tag="a"):
        lc = lhsT_c[h % 3]
        qT = lc[0:64, :]
        prhw = (pool or ps_a).tile([64, N], F32, tag=tag, name="prhw")
        prh = prhw[0:32, :]
        prw = prhw[32:64, :]
        for qh in range(32):
            sl = slice(qh * 32, qh * 32 + 32)
            nc.tensor.matmul(prh[:, sl], rhT[:, sl], qT[:, sl],
                             start=True, stop=True)
        qT3 = qT.rearrange("p (a b) -> p b a", b=32)  # [64, qw, qh]
        for qw in range(32):
            sl = slice(qw * 32, qw * 32 + 32)
            nc.tensor.matmul(prw[:, sl], rwT[:, sl], qT3[:, qw, :],
                             start=True, stop=True)
        return prhw

    def phase_C_copies(h, prhw, act=False):
        lc = lhsT_c[h % 3]
        prh = prhw[0:32, :]
        prw_v = prhw[32:64, :].rearrange("p (a b) -> p b a", b=32)
        if act:
            nc.scalar.copy(lc[64:96, 0:512], prh[:, 0:512])
            nc.scalar.copy(lc[96:128, 0:512], prw_v[:, 0:16, :])
        else:
            nc.vector.tensor_copy(lc[64:96, 0:512], prh[:, 0:512])
            nc.vector.tensor_copy(lc[96:128, 0:512], prw_v[:, 0:16, :])
        nc.vector.tensor_copy(lc[64:96, 512:1024], prh[:, 512:1024])
        nc.vector.tensor_copy(lc[96:128, 512:1024], prw_v[:, 16:32, :])

    # ---- phase B: V projection (token-major, ones column memset) ----
    def phase_B(m):
        pv = ps_av.tile([128, 6 * 64], F32, tag="av", name="pv")
        for kc in range(6):
            nc.tensor.matmul(pv[:], xT_sb[:, kc * N + m * 128:
                                          kc * N + m * 128 + 128],
                             wv_sb[:, kc * 384:kc * 384 + 384],
                             start=(kc == 0), stop=(kc == 5))
        dst = v_sb[m][:].rearrange("p (h c) -> p h c", c=65)
        if m % 2 == 0:
            nc.vector.tensor_copy(dst[:, :, 0:64],
                                  pv[:].rearrange("p (h c) -> p h c", c=64))
        else:
            nc.scalar.copy(dst[:, :, 0:64],
                           pv[:].rearrange("p (h c) -> p h c", c=64))
        nc.vector.memset(dst[:, :, 64], 1.0)

    # ---- phase D building blocks ----
    # The PE executes strictly in program order, so the emission order below
    # IS the schedule: the exp stream on Activation is the pacer (8x1038ns
    # per head) and every PE instruction is placed so its deps are satisfied
    # when its turn comes. Fillers: pqk two heads ahead, rel tables one head
    # ahead, previous head's AV(7)/normalize/transpose at the ladder top.
    pavs = {}
    norms = {}

    def S_unit(h, kb):
        lc, rc = lhsT_c[h % 3], rhs_c[h % 3]
        if kb == 7 and h >= 4:  # ps_a is idle in the last two ladders
            ps = ps_a.tile([128, N], F32, tag="a", name="s_ps")
        else:
            ps = ps_s.tile([128, N], F32, tag="s", name="s_ps")
        for half in range(2):
            sl = slice(half * 512, half * 512 + 512)
            nc.tensor.matmul(ps[:, sl], rc[:, kb * 128:kb * 128 + 128],
                             lc[:, sl], start=True, stop=True)
        if kb >= 6 and h == HPC - 1:
            # split the very last exp so the AV stop -> normalize ->
            # transpose chain (which gates the projection) starts earlier
            nc.scalar.activation(attnT[kb][:, 0:512], ps[:, 0:512],
                                 EXPF, scale=0.125)
            nc.scalar.activation(attnT[kb][:, 512:1024], ps[:, 512:1024],
                                 EXPF, scale=0.125)
        else:
            nc.scalar.activation(attnT[kb][:], ps[:], EXPF, scale=0.125)

    def AV_unit(h, kb):
        if kb == 0:
            # explicit zero + accumulate-only matmuls: interleaved start=True
            # groups within one PSUM bank corrupt earlier regions on HW
            pavs[h] = [
                ps_av.tile([128, 260], F32, tag="av", name="pav_a"),
                ps_av.tile([128, 260], F32, tag="av", name="pav_b")]
            for pv_ in pavs[h]:
                nc.tensor.matmul(pv_[:], ident, zeros[:], start=True,
                                 stop=False, skip_group_check=True)
        for qc in range(8):
            r = (qc % 4) * 65
            nc.tensor.matmul(pavs[h][qc // 4][:, r:r + 65],
                             attnT[kb][:, qc * 128:qc * 128 + 128],
                             v_sb[kb][:, h * 65:h * 65 + 65],
                             start=False, stop=(kb == 7),
                             skip_group_check=True)

    COPYF = mybir.ActivationFunctionType.Copy

    def norm_gq(h, gq, act=False):
        recip = small.tile([128, 4], F32, tag="recip", name="recip")
        norm = small.tile([128, 256], F16, tag="norm", name="norm")
        norms[(h, gq)] = norm
        dens = pavs[h][gq][:].rearrange("p (a b) -> p a b", b=65)[:, :, 64]
        nc.vector.reciprocal_approx_fast(out=recip[:], in_=dens)
        for qc4 in range(4):
            r = qc4 * 65
            dst = norm[:, qc4 * 64:qc4 * 64 + 64]
            if act and qc4 % 2 == 0:
                nc.scalar.activation(dst, pavs[h][gq][:, r:r + 64], COPYF,
                                     scale=recip[:, qc4:qc4 + 1])
            else:
                nc.vector.tensor_scalar_mul(
                    dst, pavs[h][gq][:, r:r + 64], recip[:, qc4:qc4 + 1])

    def transp_gq(h, gq, act=False):  # 4 PE transposes + copy out
        norm = norms.pop((h, gq))
        pt = ps_av.tile([64, 512], F16, tag="av", name="pt")
        for qc4 in range(4):
            nc.tensor.transpose(pt[:, qc4 * 128:qc4 * 128 + 128],
                                norm[:, qc4 * 64:qc4 * 64 + 64], ident)
        dst = proj_lhsT[h // 2][(h % 2) * 64:(h % 2) * 64 + 64,
                                gq * 512:gq * 512 + 512]
        if act:
            nc.scalar.copy(dst, pt[:])
        else:
            nc.vector.tensor_copy(dst, pt[:])

    # ---- main pipeline ----
    # The scheduler follows emission order closely; this ordering interleaves
    # next-head qk projection (chopped into token halves with immediate
    # half-drains, to keep the single ps_a slot chain short) and rel tables
    # into the ladder's early windows, leaving the exp(6)/exp(7) windows for
    # the AV tail + normalize/transpose of the current head.
    pqks = {}

    def ladder(h):
        nxt, nxt2 = h + 1 < HPC, h + 2 < HPC
        last = h == HPC - 1
        S_unit(h, 0)
        S_unit(h, 1)
        S_unit(h, 2)
        if nxt2:
            pqks[h + 2] = ps_a.tile([128, N], F32, tag="a", name="pqk")
            phase_A_half(h + 2, pqks[h + 2], 0)
            phase_A_qcopy(h + 2, pqks[h + 2], 0)
        S_unit(h, 3)
        if nxt2:
            phase_A_half(h + 2, pqks[h + 2], 1)
            phase_A_qcopy(h + 2, pqks[h + 2], 1)
            phase_A_kcopy(h + 2, pqks[h + 2], 0)
            phase_A_kcopy(h + 2, pqks[h + 2], 1)
        S_unit(h, 4)
        AV_unit(h, 0)
        prhw = phase_C_mm(h + 2) if nxt2 else None
        S_unit(h, 5)
        AV_unit(h, 1)
        if nxt2:
            phase_C_copies(h + 2, prhw)
        S_unit(h, 6)
        AV_unit(h, 2)
        S_unit(h, 7)
        AV_unit(h, 3)
        AV_unit(h, 4)
        AV_unit(h, 5)
        AV_unit(h, 6)
        AV_unit(h, 7)
        norm_gq(h, 0)
        norm_gq(h, 1)
        transp_gq(h, 0)
        transp_gq(h, 1)

    # prologue: head 0 qk (two half-token tiles in the idle S pool, so each
    # half's copies drain without waiting for the other) + rel tables, also
    # via the S pool so nothing waits on the ps_a rotation.
    pqk0h = [ps_s.tile([128, 512], F32, tag="s", name=f"pqk0{x}")
             for x in range(2)]
    for half in range(2):
        for kc in range(6):
            o = kc * N + half * 512
            nc.tensor.matmul(pqk0h[half][:], wqk_ap(kc, 0),
                             xT_sb[:, o:o + 512],
                             start=(kc == 0), stop=(kc == 5))
    for half in range(2):
        sl = slice(half * 512, half * 512 + 512)
        if with_qk_bias:
            nc.scalar.activation(lhsT_c[0][0:64, sl], pqk0h[half][0:64, :],
                                 IDENTF, bias=tbl_sb[0:64, 2176:2177])
            nc.scalar.activation(rhs_c[0][0:64, sl], pqk0h[half][64:128, :],
                                 IDENTF, bias=tbl_sb[0:64, 2177:2178])
        else:
            nc.scalar.copy(lhsT_c[0][0:64, sl], pqk0h[half][0:64, :])
            if half == 0:
                nc.vector.tensor_copy(rhs_c[0][0:64, sl],
                                      pqk0h[half][64:128, :])
            else:
                nc.scalar.copy(rhs_c[0][0:64, sl], pqk0h[half][64:128, :])
    prhw0 = phase_C_mm(0, pool=ps_s, tag="s")
    phase_C_copies(0, prhw0, act=True)

    # ladder 0 (special): B interleaved, AV(0) deferred behind B; pqk(1),
    # C(1), pqk(2) as fillers
    S_unit(0, 0)
    phase_B(0)
    S_unit(0, 1)
    phase_B(1)
    pqks[1] = ps_a.tile([128, N], F32, tag="a", name="pqk")
    phase_A_half(1, pqks[1], 0)
    phase_A_qcopy(1, pqks[1], 0)
    S_unit(0, 2)
    phase_B(2)
    phase_A_half(1, pqks[1], 1)
    phase_A_qcopy(1, pqks[1], 1)
    phase_A_kcopy(1, pqks[1], 0)
    phase_A_kcopy(1, pqks[1], 1)
    S_unit(0, 3)
    phase_B(3)
    prhw1 = phase_C_mm(1)
    S_unit(0, 4)
    phase_B(4)
    phase_C_copies(1, prhw1)
    S_unit(0, 5)
    phase_B(5)
    pqks[2] = ps_a.tile([128, N], F32, tag="a", name="pqk")
    phase_A_half(2, pqks[2], 0)
    phase_A_qcopy(2, pqks[2], 0)
    phase_A_half(2, pqks[2], 1)
    phase_A_qcopy(2, pqks[2], 1)
    phase_A_kcopy(2, pqks[2], 0)
    phase_A_kcopy(2, pqks[2], 1)
    S_unit(0, 6)
    phase_B(6)
    prhw2 = phase_C_mm(2)
    phase_C_copies(2, prhw2)
    S_unit(0, 7)
    phase_B(7)
    for kb in range(8):
        AV_unit(0, kb)
    norm_gq(0, 0)
    norm_gq(0, 1)
    transp_gq(0, 0)
    transp_gq(0, 1)

    for h in range(1, HPC):
        ladder(h)

    # ---- phase E: projection ----
    def proj_final(m, pool, tag, split):
        if split:
            pa = pool.tile([128, 512], F32, tag="av", name="pp_a")
            pb = pool.tile([128, 256], F32, tag="av", name="pp_b")
            tiles = [(pa, 0, 0, 512), (pb, 0, 512, 256)]
        else:
            pp = pool.tile([128, N], F32, tag=tag, name="pp")
            tiles = [(pp, 0, 0, 512), (pp, 512, 512, 256)]
        for t in range(3):
            for tile_, o, n0, nw in tiles:
                nc.tensor.matmul(tile_[:, o:o + nw],
                                 proj_lhsT[t][:, m * 128:m * 128 + 128],
                                 wp_sb[:, t * 768 + n0:t * 768 + n0 + nw],
                                 start=(t == 0), stop=(t == 2))
        osb = outp.tile([128, DIM], F16, tag="osb", name="osb")
        for i, (tile_, o, n0, nw) in enumerate(tiles):
            if (m + i) % 2 == 0:
                nc.vector.tensor_copy(osb[:, n0:n0 + nw], tile_[:, o:o + nw])
            else:
                nc.scalar.copy(osb[:, n0:n0 + nw], tile_[:, o:o + nw])
        eng = nc.sync if m % 2 == 0 else nc.scalar
        eng.dma_start(out_d[m * 128:m * 128 + 128, :], osb[:])

    order = [(ps_a, "a", False), (ps_s, "s", False),
             (ps_s, "s", False), (ps_av, "av", True)]
    for m in range(8):
        pool, tag, split = order[m % 4]
        proj_final(m, pool, tag, split)


def _host_prep(x, qkv_w, qkv_b, proj_w, proj_b, rel_pos_h, rel_pos_w):
    idx_h = np.arange(H)[:, None] - np.arange(H)[None, :] + (H - 1)
    idx_w = np.arange(W)[:, None] - np.arange(W)[None, :] + (W - 1)
    rhT8 = (8.0 * rel_pos_h[idx_h]).transpose(2, 0, 1).reshape(HD, H * H)
    rwT8 = (8.0 * rel_pos_w[idx_w]).transpose(2, 0, 1).reshape(HD, W * W)
    kt = np.arange(N)
    ec = np.zeros((64, N), np.float32)
    ec[:32] = (np.arange(32)[:, None] == (kt // 32)[None, :])
    ec[32:] = (np.arange(32)[:, None] == (kt % 32)[None, :])

    in_maps = []
    for core in range(NCORES):
        b = core // 2
        h0 = (core % 2) * HPC
        xb = x[b].reshape(N, DIM)
        xT_d = np.ascontiguousarray(
            xb.T.reshape(6, 128, N).transpose(1, 0, 2).reshape(128, 6 * N))

        wqk_t = np.zeros((DIM, 6 * 128), np.float32)  # [in, h*128 + (q|k)]
        wv_t = np.zeros((DIM, 6 * 64), np.float32)
        qb = np.zeros((64, 12), np.float32)
        for h in range(HPC):
            g = h0 + h
            wqk_t[:, h * 128:h * 128 + 64] = qkv_w[g * HD:(g + 1) * HD].T
            wqk_t[:, h * 128 + 64:h * 128 + 128] = \
                qkv_w[DIM + g * HD:DIM + (g + 1) * HD].T
            wv_t[:, h * 64:(h + 1) * 64] = \
                qkv_w[2 * DIM + g * HD:2 * DIM + (g + 1) * HD].T
            qb[:, 2 * h] = qkv_b[g * HD:(g + 1) * HD]
            qb[:, 2 * h + 1] = qkv_b[DIM + g * HD:DIM + (g + 1) * HD]
        # wqk packed: [p, g2*1536 + kc*256 + h2*128 + c]
        wqk_d = np.ascontiguousarray(
            wqk_t.reshape(6, 128, 3, 256).transpose(1, 2, 0, 3)
            .reshape(128, 4608))
        wv_d = np.ascontiguousarray(
            wv_t.reshape(6, 128, 384).transpose(1, 0, 2).reshape(128, 2304))

        wpm = np.zeros((HPC * HD, DIM), np.float32)
        for h in range(HPC):
            g = h0 + h
            wpm[h * HD:(h + 1) * HD, :] = proj_w[:, g * HD:(g + 1) * HD].T
        wp_d = np.ascontiguousarray(
            wpm.reshape(3, 128, 768).transpose(1, 0, 2).reshape(128, 2304))

        tbl_d = np.zeros((128, 2304), np.float32)
        tbl_d[0:64, 0:1024] = rhT8
        tbl_d[64:128, 0:1024] = rwT8
        tbl_d[0:64, 1024:2048] = ec
        tbl_d[:, 2048:2176] = np.eye(128)
        tbl_d[0:64, 2176:2188] = qb

        in_maps.append({
            "xT": xT_d.astype(np.float16),
            "wqk": wqk_d.astype(np.float16),
            "wv": wv_d.astype(np.float16),
            "wp": wp_d.astype(np.float16),
            "tbl": tbl_d.astype(np.float16),
        })
    return in_maps


def kernel(x, qkv_w, qkv_b, proj_w, proj_b, rel_pos_h, rel_pos_w, _trace=False):
    x = np.asarray(x, np.float32)
    qkv_w = np.asarray(qkv_w, np.float32)
    qkv_b = np.asarray(qkv_b, np.float32)
    proj_w = np.asarray(proj_w, np.float32)
    proj_b = np.asarray(proj_b, np.float32)
    rel_pos_h = np.asarray(rel_pos_h, np.float32)
    rel_pos_w = np.asarray(rel_pos_w, np.float32)

    in_maps = _host_prep(x, qkv_w, qkv_b, proj_w, proj_b,
                         rel_pos_h, rel_pos_w)
    with_qk_bias = bool(np.any(qkv_b[:2 * DIM]))
    key = ("nc", with_qk_bias)
    if key not in _cache:
        _cache[key] = build_program(with_qk_bias)
    nc = _cache[key]
    res = run_bass_kernel_spmd(nc, in_maps, core_ids=list(range(NCORES)),
                               trace=_trace)
    parts = [np.asarray(r["out_part"], np.float32) for r in res.results]
    pb_eff = proj_b + proj_w @ qkv_b[2 * DIM:]
    out = np.zeros((B, N, DIM), np.float32)
    for b in range(B):
        out[b] = parts[2 * b] + parts[2 * b + 1] + pb_eff
    if _trace:
        kernel.last_results = res
    return out.reshape(B, H, W, DIM)


# revision 73
# speedup vs baseline: 1.0182x; 1.0024x over previous
"""Trainium2 Bass kernel for decomposed-rel-pos attention (B=4, H=W=32, DIM=768, HEADS=12).

Sharding: 48 (batch, head) pairs -> 8 cores x 6 heads (core c: batch c//2,
heads (c%2)*6 .. +6). Each core computes qkv for its heads, attention with the
decomposed rel-pos bias folded into the S matmul as extra contraction rows
(0/1 expander matrices), softmax without max-subtraction, row-sums via a
ones-column appended to V, and a partial head-projection. Host sums the two
half-head partials per batch and adds an effective proj bias (which also
absorbs the v-bias exactly, since softmax rows sum to 1).

Numerics: all device tensors are fp16 (PE runs fp16 at 1 cycle/row with no
small-N penalty; PSUM accumulation is fp32). The softmax scale (1/8) is folded
into the exp activation's scale operand, with the rel-pos tables pre-scaled by
8 on host so the bias term comes out unscaled. End-to-end rel err vs the fp32
jax reference: ~1e-3 (tolerance 2e-2).

Layout/throughput notes (~102us/core cost-model estimate, vs 133us for the
previous f32r feature-major version):
- AV runs token-major (out [q,65] tiles, ap=65 per matmul; stationary = attnT
  slices): 4160 cycles/head + 1024 for the PE transpose back to feature-major,
  vs 8192 feature-major. Normalization becomes a per-partition scalar multiply
  (TensorScalarPtr) with one batched reciprocal per 4-chunk group.
- The pav accumulators are zeroed by a PE matmul against a zeros tile and all
  AV matmuls accumulate (start=False): interleaved start=True accumulation
  groups within one PSUM bank corrupt earlier regions on hardware.
- GPSIMD cannot access PSUM, so every PSUM-reading copy is on DVE (steady
  state) or Act (prologue/tail, where exp is not running).
- Inputs are host-packed into SBUF-layout [128, X] dram tensors so each loads
  with one large-descriptor DMA (10 input DMAs; HWDGE serialization is ~630ns
  per dma_start); xT is split in 3 so the qk projection starts early.
- Software pipeline: ladder(h) runs head h's S/exp stream (the pacer: 8 x
  ~1040ns exps on Act) and interleaves, as idle fillers: qk projection and
  rel tables for head h+2 (two ladders of slack on the single ps_a slot
  chain), the previous head's AV tail + normalize + transpose, and this
  head's AV units trailing the exps by 4. The last head's final two exps
  are split in half-token pieces so the AV-stop -> normalize -> transpose
  chain that gates the projection starts earlier.
- PSUM budget is exactly 8 banks: S pool 2x[128,1024], AV pool 2x[128,512]
  (pav accumulators then transpose targets), qk/rel-table pool 1x[128,1024].
"""
from contextlib import ExitStack

import numpy as np

import concourse.bass as bass
import concourse.bacc as bacc
import concourse.mybir as mybir
import concourse.tile as tile
from concourse.bass_utils import run_bass_kernel_spmd

B, H, W, DIM, HEADS = 4, 32, 32, 768, 12
HD = DIM // HEADS  # 64
N = H * W  # 1024
HPC = HEADS // 2  # heads per core = 6
NCORES = 8
F32 = mybir.dt.float32
F16 = mybir.dt.float16
EXPF = mybir.ActivationFunctionType.Exp
IDENTF = mybir.ActivationFunctionType.Identity

_cache = {}


def build_program(with_qk_bias=False):
    nc = bacc.Bacc("TRN2", target_bir_lowering=False, debug=False,
                   enable_asserts=False, num_devices=NCORES)
    xT = nc.dram_tensor("xT", [128, 6 * N], F16, kind="ExternalInput")
    wqk = nc.dram_tensor("wqk", [128, 6 * 768], F16, kind="ExternalInput")
    wv = nc.dram_tensor("wv", [128, 6 * 384], F16, kind="ExternalInput")
    wp = nc.dram_tensor("wp", [128, 3 * 768], F16, kind="ExternalInput")
    tbl = nc.dram_tensor("tbl", [128, 2304], F16, kind="ExternalInput")
    out_d = nc.dram_tensor("out_part", [N, DIM], F16, kind="ExternalOutput")

    with ExitStack() as ctx:
        tc = ctx.enter_context(tile.TileContext(nc))
        _body(nc, tc, ctx, xT, wqk, wv, wp, tbl, out_d, with_qk_bias)
    nc.compile()
    return nc


def _body(nc, tc, ctx, xT, wqk, wv, wp, tbl, out_d, with_qk_bias):
    persist = ctx.enter_context(tc.tile_pool(name="persist", bufs=1))
    attn_pool = ctx.enter_context(tc.tile_pool(name="attn", bufs=1))
    small = ctx.enter_context(tc.tile_pool(name="small", bufs=2))
    outp = ctx.enter_context(tc.tile_pool(name="outp", bufs=6))
    ps_s = ctx.enter_context(tc.tile_pool(name="ps_s", bufs=2, space="PSUM"))
    ps_av = ctx.enter_context(tc.tile_pool(name="ps_av", bufs=2, space="PSUM"))
    ps_a = ctx.enter_context(tc.tile_pool(name="ps_a", bufs=1, space="PSUM"))

    # ---- persistent SBUF tiles ----
    xT_sb = persist.tile([128, 6 * N], F16, tag="xt", name="xt")
    wqk_sb = persist.tile([128, 6 * 768], F16, tag="wqk", name="wqk")
    wv_sb = persist.tile([128, 6 * 384], F16, tag="wv", name="wv")
    wp_sb = persist.tile([128, 3 * 768], F16, tag="wp", name="wp")
    tbl_sb = persist.tile([128, 2304], F16, tag="tbl", name="tbl")
    rhT = tbl_sb[0:64, 0:1024]
    rwT = persist.tile([64, 1024], F16, tag="rwT", name="rwT")
    ecomb = tbl_sb[0:64, 1024:2048]
    ident = tbl_sb[:, 2048:2176]
    v_sb = [persist.tile([128, HPC * 65], F16, tag=f"v{m}", name=f"v{m}")
            for m in range(8)]
    proj_lhsT = [persist.tile([128, N], F16, tag=f"pl{t}", name=f"pl{t}")
                 for t in range(3)]
    lhsT_c = [persist.tile([128, N], F16, tag=f"lhs{i}", name=f"lhs{i}")
              for i in range(3)]
    rhs_c = [persist.tile([128, N], F16, tag=f"rhs{i}", name=f"rhs{i}")
             for i in range(3)]
    attnT = [attn_pool.tile([128, N], F16, tag=f"attnT{kb}", name=f"attnT{kb}")
             for kb in range(8)]
    zeros = persist.tile([128, 260], F16, tag="zeros", name="zeros")
    nc.vector.memset(zeros[:], 0.0)
    osb_all = persist.tile([128, 8 * DIM], F16, tag="osb", name="osb_all")

    # ---- input DMAs: two queues, interleaved; each is one large transfer ----
    # wqk is packed by head-pair group g: col g*1536 + kc*256 + (h%2)*128 + c,
    # so group g arrives early enough to gate only heads 2g, 2g+1.
    nc.sync.dma_start(xT_sb[:, 0:N], xT[:, 0:N])
    nc.sync.dma_start(xT_sb[:, N:3 * N], xT[:, N:3 * N])
    nc.scalar.dma_start(wqk_sb[:, 0:1536], wqk[:, 0:1536])
    nc.scalar.dma_start(xT_sb[:, 3 * N:6 * N], xT[:, 3 * N:6 * N])
    nc.scalar.dma_start(tbl_sb[:, 0:2048], tbl[:, 0:2048])
    nc.scalar.dma_start(tbl_sb[:, 2048:2304], tbl[:, 2048:2304])
    nc.scalar.dma_start(wv_sb[:], wv[:])
    nc.scalar.dma_start(wqk_sb[:, 1536:3072], wqk[:, 1536:3072])
    nc.scalar.dma_start(wqk_sb[:, 3072:4608], wqk[:, 3072:4608])
    nc.scalar.dma_start(wp_sb[:], wp[:])

    # ecomb rows into both rhs buffers once (rows 64:128 never rewritten);
    # rwT to a partition-0 tile (PE matmul needs matching base partitions)
    nc.vector.tensor_copy(rwT[:], tbl_sb[64:128, 0:1024])
    nc.vector.tensor_copy(rhs_c[0][64:128, :], ecomb)
    nc.vector.tensor_copy(rhs_c[1][64:128, :], ecomb)
    nc.vector.tensor_copy(rhs_c[2][64:128, :], ecomb)

    def wqk_ap(kc, h):
        g, h2 = h // 2, h % 2
        o = g * 1536 + kc * 256 + h2 * 128
        return wqk_sb[:, o:o + 128]

    # ---- phase A: per-head qk projection [q64|k64 rows, tok] ----
    def phase_A_half(h, pqk, half):
        sl = slice(half * 512, half * 512 + 512)
        for kc in range(6):
            nc.tensor.matmul(pqk[:, sl], wqk_ap(kc, h),
                             xT_sb[:, kc * N + half * 512:
                                   kc * N + half * 512 + 512],
                             start=(kc == 0), stop=(kc == 5))

    def phase_A_mm(h):
        pqk = ps_a.tile([128, N], F32, tag="a", name="pqk")
        phase_A_half(h, pqk, 0)
        phase_A_half(h, pqk, 1)
        return pqk

    def phase_A_qcopy(h, pqk, half):
        lc = lhsT_c[h % 3]
        sl = slice(half * 512, half * 512 + 512)
        if with_qk_bias:
            nc.scalar.activation(lc[0:64, sl], pqk[0:64, sl], IDENTF,
                                 bias=tbl_sb[0:64, 2176 + 2 * h:2177 + 2 * h])
        else:
            nc.vector.tensor_copy(lc[0:64, sl], pqk[0:64, sl])

    def phase_A_kcopy(h, pqk, half):
        rc = rhs_c[h % 3]
        sl = slice(half * 512, half * 512 + 512)
        if with_qk_bias:
            nc.scalar.activation(rc[0:64, sl], pqk[64:128, sl], IDENTF,
                                 bias=tbl_sb[0:64, 2177 + 2 * h:2178 + 2 * h])
        else:
            nc.vector.tensor_copy(rc[0:64, sl], pqk[64:128, sl])

    def phase_A_copies(h, pqk):
        phase_A_qcopy(h, pqk, 0)
        phase_A_qcopy(h, pqk, 1)
        phase_A_kcopy(h, pqk, 0)
        phase_A_kcopy(h, pqk, 1)

    # ---- phase C: rel-pos tables -> bias rows of lhsT_c ----
    # prh/prw share one [64, N] psum tile (partitions 0:32 / 32:64) from the
    # ps_a pool, sequenced after pqk's drain, so this phase never waits on
    # the exp-paced S-pool slots.
    def phase_C_mm(h, pool=None, tag="a"):
        lc = lhsT_c[h % 3]
        qT = lc[0:64, :]
        prhw = (pool or ps_a).tile([64, N], F32, tag=tag, name="prhw")
        prh = prhw[0:32, :]
        prw = prhw[32:64, :]
        for qh in range(32):
            sl = slice(qh * 32, qh * 32 + 32)
            nc.tensor.matmul(prh[:, sl], rhT[:, sl], qT[:, sl],
                             start=True, stop=True)
        qT3 = qT.rearrange("p (a b) -> p b a", b=32)  # [64, qw, qh]
        for qw in range(32):
            sl = slice(qw * 32, qw * 32 + 32)
            nc.tensor.matmul(prw[:, sl], rwT[:, sl], qT3[:, qw, :],
                             start=True, stop=True)
        return prhw

    def phase_C_copies(h, prhw, act=False):
        lc = lhsT_c[h % 3]
        prh = prhw[0:32, :]
        prw_v = prhw[32:64, :].rearrange("p (a b) -> p b a", b=32)
        if act:
            nc.scalar.copy(lc[64:96, 0:512], prh[:, 0:512])
            nc.scalar.copy(lc[96:128, 0:512], prw_v[:, 0:16, :])
        else:
            nc.vector.tensor_copy(lc[64:96, 0:512], prh[:, 0:512])
            nc.vector.tensor_copy(lc[96:128, 0:512], prw_v[:, 0:16, :])
        nc.vector.tensor_copy(lc[64:96, 512:1024], prh[:, 512:1024])
        nc.vector.tensor_copy(lc[96:128, 512:1024], prw_v[:, 16:32, :])

    # ---- phase B: V projection (token-major, ones column memset) ----
    def phase_B(m):
        pv = ps_av.tile([128, 6 * 64], F32, tag="av", name="pv")
        for kc in range(6):
            nc.tensor.matmul(pv[:], xT_sb[:, kc * N + m * 128:
                                          kc * N + m * 128 + 128],
                             wv_sb[:, kc * 384:kc * 384 + 384],
                             start=(kc == 0), stop=(kc == 5))
        dst = v_sb[m][:].rearrange("p (h c) -> p h c", c=65)
        if m % 2 == 0:
            nc.vector.tensor_copy(dst[:, :, 0:64],
                                  pv[:].rearrange("p (h c) -> p h c", c=64))
        else:
            nc.scalar.copy(dst[:, :, 0:64],
                           pv[:].rearrange("p (h c) -> p h c", c=64))
        nc.vector.memset(dst[:, :, 64], 1.0)

    # ---- phase D building blocks ----
    # The PE executes strictly in program order, so the emission order below
    # IS the schedule: the exp stream on Activation is the pacer (8x1038ns
    # per head) and every PE instruction is placed so its deps are satisfied
    # when its turn comes. Fillers: pqk two heads ahead, rel tables one head
    # ahead, previous head's AV(7)/normalize/transpose at the ladder top.
    pavs = {}
    norms = {}

    def S_unit(h, kb):
        lc, rc = lhsT_c[h % 3], rhs_c[h % 3]
        if kb == 7 and h >= 4:  # ps_a is idle in the last two ladders
            ps = ps_a.tile([128, N], F32, tag="a", name="s_ps")
        else:
            ps = ps_s.tile([128, N], F32, tag="s", name="s_ps")
        for half in range(2):
            sl = slice(half * 512, half * 512 + 512)
            nc.tensor.matmul(ps[:, sl], rc[:, kb * 128:kb * 128 + 128],
                             lc[:, sl], start=True, stop=True)
        if kb >= 6 and h == HPC - 1:
            # split the very last exp so the AV stop -> normalize ->
            # transpose chain (which gates the projection) starts earlier
            nc.scalar.activation(attnT[kb][:, 0:512], ps[:, 0:512],
                                 EXPF, scale=0.125)
            nc.scalar.activation(attnT[kb][:, 512:1024], ps[:, 512:1024],
                                 EXPF, scale=0.125)
        else:
            nc.scalar.activation(attnT[kb][:], ps[:], EXPF, scale=0.125)

    def AV_unit(h, kb):
        if kb == 0:
            # explicit zero + accumulate-only matmuls: interleaved start=True
            # groups within one PSUM bank corrupt earlier regions on HW
            pavs[h] = [
                ps_av.tile([128, 260], F32, tag="av", name="pav_a"),
                ps_av.tile([128, 260], F32, tag="av", name="pav_b")]
            for pv_ in pavs[h]:
                nc.tensor.matmul(pv_[:], ident, zeros[:], start=True,
                                 stop=False, skip_group_check=True)
        for qc in range(8):
            r = (qc % 4) * 65
            nc.tensor.matmul(pavs[h][qc // 4][:, r:r + 65],
                             attnT[kb][:, qc * 128:qc * 128 + 128],
                             v_sb[kb][:, h * 65:h * 65 + 65],
                             start=False, stop=(kb == 7),
                             skip_group_check=True)

    COPYF = mybir.ActivationFunctionType.Copy

    def norm_gq(h, gq, act=False):
        recip = small.tile([128, 4], F32, tag="recip", name="recip")
        norm = small.tile([128, 256], F16, tag="norm", name="norm")
        norms[(h, gq)] = norm
        dens = pavs[h][gq][:].rearrange("p (a b) -> p a b", b=65)[:, :, 64]
        nc.vector.reciprocal_approx_fast(out=recip[:], in_=dens)
        for qc4 in range(4):
            r = qc4 * 65
            dst = norm[:, qc4 * 64:qc4 * 64 + 64]
            if act and qc4 % 2 == 0:
                nc.scalar.activation(dst, pavs[h][gq][:, r:r + 64], COPYF,
                                     scale=recip[:, qc4:qc4 + 1])
            else:
                nc.vector.tensor_scalar_mul(
                    dst, pavs[h][gq][:, r:r + 64], recip[:, qc4:qc4 + 1])

    def transp_gq(h, gq, act=False):  # 4 PE transposes + copy out
        norm = norms.pop((h, gq))
        pt = ps_av.tile([64, 512], F16, tag="av", name="pt")
        for qc4 in range(4):
            nc.tensor.transpose(pt[:, qc4 * 128:qc4 * 128 + 128],
                                norm[:, qc4 * 64:qc4 * 64 + 64], ident)
        dst = proj_lhsT[h // 2][(h % 2) * 64:(h % 2) * 64 + 64,
                                gq * 512:gq * 512 + 512]
        if act:
            nc.scalar.copy(dst, pt[:])
        else:
            nc.vector.tensor_copy(dst, pt[:])

    # ---- main pipeline ----
    # The scheduler follows emission order closely; this ordering interleaves
    # next-head qk projection (chopped into token halves with immediate
    # half-drains, to keep the single ps_a slot chain short) and rel tables
    # into the ladder's early windows, leaving the exp(6)/exp(7) windows for
    # the AV tail + normalize/transpose of the current head.
    pqks = {}

    def ladder(h):
        nxt, nxt2 = h + 1 < HPC, h + 2 < HPC
        last = h == HPC - 1
        S_unit(h, 0)
        S_unit(h, 1)
        S_unit(h, 2)
        if nxt2:
            pqks[h + 2] = ps_a.tile([128, N], F32, tag="a", name="pqk")
            phase_A_half(h + 2, pqks[h + 2], 0)
            phase_A_qcopy(h + 2, pqks[h + 2], 0)
        S_unit(h, 3)
        if nxt2:
            phase_A_half(h + 2, pqks[h + 2], 1)
            phase_A_qcopy(h + 2, pqks[h + 2], 1)
            phase_A_kcopy(h + 2, pqks[h + 2], 0)
            phase_A_kcopy(h + 2, pqks[h + 2], 1)
        S_unit(h, 4)
        AV_unit(h, 0)
        prhw = phase_C_mm(h + 2) if nxt2 else None
        S_unit(h, 5)
        AV_unit(h, 1)
        if nxt2:
            phase_C_copies(h + 2, prhw)
        S_unit(h, 6)
        AV_unit(h, 2)
        S_unit(h, 7)
        AV_unit(h, 3)
        AV_unit(h, 4)
        AV_unit(h, 5)
        AV_unit(h, 6)
        AV_unit(h, 7)
        norm_gq(h, 0)
        norm_gq(h, 1)
        transp_gq(h, 0)
        transp_gq(h, 1)

    # prologue: head 0 qk (two half-token tiles in the idle S pool, so each
    # half's copies drain without waiting for the other) + rel tables, also
    # via the S pool so nothing waits on the ps_a rotation.
    pqk0h = [ps_s.tile([128, 512], F32, tag="s", name=f"pqk0{x}")
             for x in range(2)]
    for half in range(2):
        for kc in range(6):
            o = kc * N + half * 512
            nc.tensor.matmul(pqk0h[half][:], wqk_ap(kc, 0),
                             xT_sb[:, o:o + 512],
                             start=(kc == 0), stop=(kc == 5))
    for half in range(2):
        sl = slice(half * 512, half * 512 + 512)
        if with_qk_bias:
            nc.scalar.activation(lhsT_c[0][0:64, sl], pqk0h[half][0:64, :],
                                 IDENTF, bias=tbl_sb[0:64, 2176:2177])
            nc.scalar.activation(rhs_c[0][0:64, sl], pqk0h[half][64:128, :],
                                 IDENTF, bias=tbl_sb[0:64, 2177:2178])
        else:
            nc.scalar.copy(lhsT_c[0][0:64, sl], pqk0h[half][0:64, :])
            if half == 0:
                nc.vector.tensor_copy(rhs_c[0][0:64, sl],
                                      pqk0h[half][64:128, :])
            else:
                nc.scalar.copy(rhs_c[0][0:64, sl], pqk0h[half][64:128, :])
    prhw0 = phase_C_mm(0, pool=ps_s, tag="s")
    phase_C_copies(0, prhw0, act=True)

    # ladder 0 (special): B interleaved, AV(0) deferred behind B; pqk(1),
    # C(1), pqk(2) as fillers
    S_unit(0, 0)
    phase_B(0)
    S_unit(0, 1)
    phase_B(1)
    pqks[1] = ps_a.tile([128, N], F32, tag="a", name="pqk")
    phase_A_half(1, pqks[1], 0)
    phase_A_qcopy(1, pqks[1], 0)
    S_unit(0, 2)
    phase_B(2)
    phase_A_half(1, pqks[1], 1)
    phase_A_qcopy(1, pqks[1], 1)
    phase_A_kcopy(1, pqks[1], 0)
    phase_A_kcopy(1, pqks[1], 1)
    S_unit(0, 3)
    phase_B(3)
    prhw1 = phase_C_mm(1)
    S_unit(0, 4)
    phase_B(4)
    phase_C_copies(1, prhw1)
    S_unit(0, 5)
    phase_B(5)
    pqks[2] = ps_a.tile([128, N], F32, tag="a", name="pqk")
    phase_A_half(2, pqks[2], 0)
    phase_A_qcopy(2, pqks[2], 0)
    phase_A_half(2, pqks[2], 1)
    phase_A_qcopy(2, pqks[2], 1)
    phase_A_kcopy(2, pqks[2], 0)
    phase_A_kcopy(2, pqks[2], 1)
    S_unit(0, 6)
    phase_B(6)
    prhw2 = phase_C_mm(2)
    phase_C_copies(2, prhw2)
    S_unit(0, 7)
    phase_B(7)
    for kb in range(8):
        AV_unit(0, kb)
    norm_gq(0, 0)
    norm_gq(0, 1)
    transp_gq(0, 0)
    transp_gq(0, 1)

    for h in range(1, HPC):
        ladder(h)

    # ---- phase E: projection ----
    def proj_final(m, pool, tag, split):
        if split:
            pa = pool.tile([128, 512], F32, tag="av", name="pp_a")
            pb = pool.tile([128, 256], F32, tag="av", name="pp_b")
            tiles = [(pa, 0, 0, 512), (pb, 0, 512, 256)]
        else:
            pp = pool.tile([128, N], F32, tag=tag, name="pp")
            tiles = [(pp, 0, 0, 512), (pp, 512, 512, 256)]
        for t in range(3):
            for tile_, o, n0, nw in tiles:
                nc.tensor.matmul(tile_[:, o:o + nw],
                                 proj_lhsT[t][:, m * 128:m * 128 + 128],
                                 wp_sb[:, t * 768 + n0:t * 768 + n0 + nw],
                                 start=(t == 0), stop=(t == 2))
        ob = m * DIM
        for i, (tile_, o, n0, nw) in enumerate(tiles):
            if (m + i) % 2 == 0:
                nc.vector.tensor_copy(osb_all[:, ob + n0:ob + n0 + nw],
                                      tile_[:, o:o + nw])
            else:
                nc.scalar.copy(osb_all[:, ob + n0:ob + n0 + nw],
                               tile_[:, o:o + nw])
        if m % 2 == 1:
            src_ap = osb_all[:, (m - 1) * DIM:(m + 1) * DIM].rearrange(
                "p (b c) -> p b c", b=2)
            dst_ap = out_d[(m - 1) * 128:(m + 1) * 128, :].rearrange(
                "(b p) c -> p b c", p=128)
            eng = nc.sync if (m // 2) % 2 == 0 else nc.scalar
            eng.dma_start(dst_ap, src_ap)

    order = [(ps_a, "a", False), (ps_s, "s", False),
             (ps_s, "s", False), (ps_av, "av", True)]
    for m in range(8):
        pool, tag, split = order[m % 4]
        proj_final(m, pool, tag, split)


def _host_prep(x, qkv_w, qkv_b, proj_w, proj_b, rel_pos_h, rel_pos_w):
    idx_h = np.arange(H)[:, None] - np.arange(H)[None, :] + (H - 1)
    idx_w = np.arange(W)[:, None] - np.arange(W)[None, :] + (W - 1)
    rhT8 = (8.0 * rel_pos_h[idx_h]).transpose(2, 0, 1).reshape(HD, H * H)
    rwT8 = (8.0 * rel_pos_w[idx_w]).transpose(2, 0, 1).reshape(HD, W * W)
    kt = np.arange(N)
    ec = np.zeros((64, N), np.float32)
    ec[:32] = (np.arange(32)[:, None] == (kt // 32)[None, :])
    ec[32:] = (np.arange(32)[:, None] == (kt % 32)[None, :])

    in_maps = []
    for core in range(NCORES):
        b = core // 2
        h0 = (core % 2) * HPC
        xb = x[b].reshape(N, DIM)
        xT_d = np.ascontiguousarray(
            xb.T.reshape(6, 128, N).transpose(1, 0, 2).reshape(128, 6 * N))

        wqk_t = np.zeros((DIM, 6 * 128), np.float32)  # [in, h*128 + (q|k)]
        wv_t = np.zeros((DIM, 6 * 64), np.float32)
        qb = np.zeros((64, 12), np.float32)
        for h in range(HPC):
            g = h0 + h
            wqk_t[:, h * 128:h * 128 + 64] = qkv_w[g * HD:(g + 1) * HD].T
            wqk_t[:, h * 128 + 64:h * 128 + 128] = \
                qkv_w[DIM + g * HD:DIM + (g + 1) * HD].T
            wv_t[:, h * 64:(h + 1) * 64] = \
                qkv_w[2 * DIM + g * HD:2 * DIM + (g + 1) * HD].T
            qb[:, 2 * h] = qkv_b[g * HD:(g + 1) * HD]
            qb[:, 2 * h + 1] = qkv_b[DIM + g * HD:DIM + (g + 1) * HD]
        # wqk packed: [p, g2*1536 + kc*256 + h2*128 + c]
        wqk_d = np.ascontiguousarray(
            wqk_t.reshape(6, 128, 3, 256).transpose(1, 2, 0, 3)
            .reshape(128, 4608))
        wv_d = np.ascontiguousarray(
            wv_t.reshape(6, 128, 384).transpose(1, 0, 2).reshape(128, 2304))

        wpm = np.zeros((HPC * HD, DIM), np.float32)
        for h in range(HPC):
            g = h0 + h
            wpm[h * HD:(h + 1) * HD, :] = proj_w[:, g * HD:(g + 1) * HD].T
        wp_d = np.ascontiguousarray(
            wpm.reshape(3, 128, 768).transpose(1, 0, 2).reshape(128, 2304))

        tbl_d = np.zeros((128, 2304), np.float32)
        tbl_d[0:64, 0:1024] = rhT8
        tbl_d[64:128, 0:1024] = rwT8
        tbl_d[0:64, 1024:2048] = ec
        tbl_d[:, 2048:2176] = np.eye(128)
        tbl_d[0:64, 2176:2188] = qb

        in_maps.append({
            "xT": xT_d.astype(np.float16),
            "wqk": wqk_d.astype(np.float16),
            "wv": wv_d.astype(np.float16),
            "wp": wp_d.astype(np.float16),
            "tbl": tbl_d.astype(np.float16),
        })
    return in_maps


def kernel(x, qkv_w, qkv_b, proj_w, proj_b, rel_pos_h, rel_pos_w, _trace=False):
    x = np.asarray(x, np.float32)
    qkv_w = np.asarray(qkv_w, np.float32)
    qkv_b = np.asarray(qkv_b, np.float32)
    proj_w = np.asarray(proj_w, np.float32)
    proj_b = np.asarray(proj_b, np.float32)
    rel_pos_h = np.asarray(rel_pos_h, np.float32)
    rel_pos_w = np.asarray(rel_pos_w, np.float32)

    in_maps = _host_prep(x, qkv_w, qkv_b, proj_w, proj_b,
                         rel_pos_h, rel_pos_w)
    with_qk_bias = bool(np.any(qkv_b[:2 * DIM]))
    key = ("nc", with_qk_bias)
    if key not in _cache:
        _cache[key] = build_program(with_qk_bias)
    nc = _cache[key]
    res = run_bass_kernel_spmd(nc, in_maps, core_ids=list(range(NCORES)),
                               trace=_trace)
    parts = [np.asarray(r["out_part"], np.float32) for r in res.results]
    pb_eff = proj_b + proj_w @ qkv_b[2 * DIM:]
    out = np.zeros((B, N, DIM), np.float32)
    for b in range(B):
        out[b] = parts[2 * b] + parts[2 * b + 1] + pb_eff
    if _trace:
        kernel.last_results = res
    return out.reshape(B, H, W, DIM)
